# revision 1
# baseline (speedup 1.0000x reference)
"""GQA attention (B=1, S=2048, H=2048, 32 q-heads / 8 kv-heads, hd=64)
on 8 Trainium2 NeuronCores.

Sharding: tensor-parallel over heads. Core c owns q-heads 4c..4c+3 and
kv-head c: wq/wk/wv column shards, wo row shard; each core computes a
full [S, H] partial of the output projection; chunked ReduceScatters
(256 output rows each, overlapped with compute) sum the partials; the
host scatters the per-core slices back together.

Device program (per core), all matmuls fp32r (~bf16 rate, ~1e-4 rel):
  phase AB, pipelined per 1024-column half:
    qT/kT/vT projections (weights host-pretiled for contiguous DMA),
    per-head RMSNorm (ones-block matmul partition sums; rstd via ACT
    Ln -> Exp(-0.5x), table switches batched), RoPE via partition-
    shifted sbuf-sbuf DMA + 3 DVE ops, V transposed on TensorE into
    V_aug with a ones column.
  phase CDE per 512-wide q-chunk:
    scores^T [128 kpos, q] = kT-tile.T @ qT, 2 heads row-packed into
    one [128,1024] psum, P^T = exp(0.125 S^T) on ScalarE (bounded
    scores: no max pass), causal mask on the diagonal 128x128 block,
    attnT_aug [65, q] += V_aug.T @ P^T (ones column -> l),
    normalize by 1/l = Exp(-Ln(l)) broadcast via ones-block matmul,
    o_proj per 128-row tile, ReduceScatter every 256 rows.
"""
import os
import sys

sys.path.insert(0, "/opt/trn_rl_repo")

import numpy as np  # noqa: E402
import concourse.bacc as bacc  # noqa: E402
import concourse.mybir as mybir  # noqa: E402
import concourse.tile as tile  # noqa: E402
from concourse import bass_utils  # noqa: E402

f32 = mybir.dt.float32
f32r = mybir.dt.float32r
bf16 = mybir.dt.bfloat16
AF = mybir.ActivationFunctionType

N_CORES = 8
S = 2048
HID = 2048
HD = 64
ROPE_THETA = 10000.0
RMS_EPS = 1e-6
SCALING = HD ** -0.5              # 0.125
NK = HID // 128                   # 16 contraction tiles
NQC = S // 512                    # 4 q chunks
NKT = S // 128                    # 16 kpos tiles
RS_BF16 = os.environ.get("KRS16", "0") == "1"

_NC_CACHE = None
LAST_RESULTS = None


def _build():
    nc = bacc.Bacc("TRN2", target_bir_lowering=False, debug=False,
                   num_devices=N_CORES)

    def din(name, shape, dt):
        return nc.dram_tensor(name, shape, dt, kind="ExternalInput").ap()

    xT = din("xT", [HID, S], f32r)
    # host-pretiled: row p, col block t = original rows 128t+p
    wq0 = din("wq0", [128, HID], f32r)
    wq1 = din("wq1", [128, HID], f32r)
    wkv = din("wkv", [128, HID], f32r)     # [wv | wk] columns pretiled
    wo0 = din("wo0", [128, S], f32r)
    wo1 = din("wo1", [128, S], f32r)
    cos2 = din("cos2", [128, S], f32)
    ss2 = din("ss2", [128, S], f32)
    ew_q = din("ew_q", [2, 128], f32r)
    ew_k = din("ew_k", [2, 128], f32r)
    e2 = din("e2", [2, 128], f32r)
    e2t = din("e2t", [128, 2], f32r)
    mask = din("mask", [128, 128], f32r)
    ident = din("ident", [64, 64], f32)

    out_rs = nc.dram_tensor("out_rs", [S // N_CORES, S], f32,
                            kind="ExternalOutput").ap()

    rs_dt = bf16 if RS_BF16 else f32

    with tile.TileContext(nc) as tc:
        with tc.tile_pool(name="consts", bufs=1) as cp, \
             tc.tile_pool(name="dram", bufs=1, space="DRAM") as dp:
            c_wq0 = cp.tile([128, HID], f32r, tag="w")
            c_wq1 = cp.tile([128, HID], f32r, tag="w2")
            c_wkv = cp.tile([128, HID], f32r, tag="w3")
            c_wo0 = cp.tile([128, S], f32r, tag="w4")
            c_wo1 = cp.tile([128, S], f32r, tag="w5")
            c_cos = cp.tile([128, S], f32, tag="c1")
            c_ss = cp.tile([128, S], f32, tag="c2")
            c_ewq = cp.tile([2, 128], f32r, tag="c3")
            c_ewk = cp.tile([2, 128], f32r, tag="c4")
            c_e2 = cp.tile([2, 128], f32r, tag="c5")
            c_e2t = cp.tile([128, 2], f32r, tag="c5t")
            c_mask = cp.tile([128, 128], f32r, tag="c6")
            c_id = cp.tile([64, 64], f32, tag="c7")
            c_eps = cp.tile([2, 1], f32, tag="c8")

            # phase-A weights first (contiguous, 8KB rows);
            # wq0 complete first so the first matmul unblocks early
            for dst_t, src_t in ((c_wq0, wq0), (c_wq1, wq1),
                                 (c_wkv, wkv)):
                for h in range(4):
                    hr = slice(32 * h, 32 * h + 32)
                    nc.sync.dma_start(dst_t[hr, :], src_t[hr, :])
            nc.vector.memset(c_eps[:], RMS_EPS)
            nc.sync.dma_start(c_e2t[:], e2t)
            nc.sync.dma_start(c_ewq[:], ew_q)
            nc.sync.dma_start(c_ewk[:], ew_k)
            nc.sync.dma_start(c_id[:], ident)
            nc.sync.dma_start(c_cos[:], cos2)
            nc.sync.dma_start(c_ss[:], ss2)

            qkv = {
                "q0": cp.tile([128, S], f32, tag="q0", name="q0"),
                "q1": cp.tile([128, S], f32, tag="q1", name="q1"),
                "kv": cp.tile([128, S], f32, tag="kv", name="kv"),
            }
            qr0 = cp.tile([128, S], f32r, tag="qr0")
            qr1 = cp.tile([128, S], f32r, tag="qr1")
            krd = cp.tile([128, S], f32r, tag="krd")
            v_aug = cp.tile([128, NKT * (HD + 1)], f32r, tag="vaug")

            attn_raw = [cp.tile([128, S], f32, tag=f"araw{i}",
                                name=f"araw{i}") for i in range(2)]
            l_sb = [cp.tile([2, S], f32, tag=f"l{i}", name=f"l{i}")
                    for i in range(2)]

            partial = dp.tile([S, S], rs_dt)
            rs_out = dp.tile([S // N_CORES, S], rs_dt)

            # ---- Phase A+B pipelined per 1024-col half ----
            with tc.tile_pool(name="xt", bufs=4) as xp, \
                 tc.tile_pool(name="sbB", bufs=2) as sbB, \
                 tc.tile_pool(name="psA", bufs=3, space="PSUM") as psA, \
                 tc.tile_pool(name="psM", bufs=2, space="PSUM") as psM:
                specs = [
                    ("kv", c_ewk, krd, True),
                    ("q0", c_ewq, qr0, False),
                    ("q1", c_ewq, qr1, False),
                ]
                for qh in range(2):
                    hs = slice(1024 * qh, 1024 * qh + 1024)
                    # --- A: projections for this half ---
                    pq = [psA.tile([128, 1024], f32, tag="pa",
                                   name=f"pa{qh}_{j}") for j in range(3)]
                    for t in range(NK):
                        xt = xp.tile([128, 1024], f32r, tag="xt")
                        for h in range(2):
                            hr = slice(64 * h, 64 * h + 64)
                            nc.sync.dma_start(
                                xt[hr, :],
                                xT[128 * t + 64 * h:128 * t + 64 * h + 64,
                                   hs])
                        st = (t == 0)
                        sp = (t == NK - 1)
                        tc_ = slice(128 * t, 128 * (t + 1))
                        for j, w in ((0, c_wq0), (1, c_wq1), (2, c_wkv)):
                            nc.tensor.matmul(pq[j][:, 0:512], w[:, tc_],
                                             xt[:, 0:512],
                                             start=st, stop=sp)
                            nc.tensor.matmul(pq[j][:, 512:1024], w[:, tc_],
                                             xt[:, 512:1024],
                                             start=st, stop=sp)
                    for j, key in ((0, "q0"), (1, "q1"), (2, "kv")):
                        nc.vector.tensor_copy(qkv[key][:, hs], pq[j][:])

                    # --- B: norm + rope for the two 512-chunks ---
                    # stats: Ln batch then Exp batch (2 table switches)
                    lnvs = {}
                    for si, (key, ew, dst, is_kv) in enumerate(specs):
                        src = qkv[key]
                        sq = sbB.tile([128, 1024], f32r, tag="sq",
                                      bufs=2, name=f"sq{qh}_{si}")
                        nc.vector.tensor_mul(sq[:], src[:, hs], src[:, hs])
                        for u in range(2):
                            us = slice(512 * u, 512 * u + 512)
                            pss = psM.tile([2, 512], f32, tag="m",
                                           name=f"ss{qh}_{si}_{u}")
                            nc.tensor.matmul(pss[:], c_e2t[:], sq[:, us],
                                             start=True, stop=True)
                            lnv = sbB.tile([2, 512], f32, tag="lnv",
                                           bufs=6, name=f"lnv{qh}{si}{u}")
                            nc.scalar.activation(lnv[:], pss[:], AF.Ln,
                                                 scale=1.0 / HD,
                                                 bias=c_eps[:])
                            lnvs[(si, u)] = lnv
                    rstds = {}
                    for si in range(3):
                        for u in range(2):
                            rr = sbB.tile([2, 512], f32r, tag="rstdr",
                                          bufs=6, name=f"rr{qh}{si}{u}")
                            nc.scalar.activation(rr[:], lnvs[(si, u)][:],
                                                 AF.Exp, scale=-0.5)
                            rstds[(si, u)] = rr
                    for si, (key, ew, dst, is_kv) in enumerate(specs):
                        src = qkv[key]
                        rows = slice(64, 128) if is_kv else slice(0, 128)
                        nrm = sbB.tile([128, 1024], f32, tag="nrm",
                                       bufs=2, name=f"nrm{qh}_{si}")
                        for u in range(2):
                            cs = slice(1024 * qh + 512 * u,
                                       1024 * qh + 512 * u + 512)
                            us = slice(512 * u, 512 * u + 512)
                            pb = psM.tile([128, 512], f32, tag="m",
                                          name=f"pb{qh}_{si}_{u}")
                            nc.tensor.matmul(pb[:], ew[:],
                                             rstds[(si, u)][:],
                                             start=True, stop=True)
                            nc.vector.tensor_mul(nrm[rows, us],
                                                 src[rows, cs],
                                                 pb[rows, :])
                        # rope
                        sh = sbB.tile([128, 1024], f32, tag="sh",
                                      bufs=2, name=f"sh{qh}_{si}")
                        if is_kv:
                            nc.sync.dma_start(sh[64:96, :], nrm[96:128, :])
                            nc.sync.dma_start(sh[96:128, :], nrm[64:96, :])
                        else:
                            nc.sync.dma_start(sh[0:32, :], nrm[32:64, :])
                            nc.sync.dma_start(sh[32:64, :], nrm[0:32, :])
                            nc.sync.dma_start(sh[64:96, :], nrm[96:128, :])
                            nc.sync.dma_start(sh[96:128, :], nrm[64:96, :])
                        t2 = sbB.tile([128, 1024], f32, tag="sq",
                                      bufs=2, name=f"t2{qh}_{si}")
                        nc.vector.tensor_mul(t2[rows, :], sh[rows, :],
                                             c_ss[rows, hs])
                        t1 = sbB.tile([128, 1024], f32, tag="sh",
                                      bufs=2, name=f"t1{qh}_{si}")
                        nc.vector.tensor_mul(t1[rows, :], nrm[rows, :],
                                             c_cos[rows, hs])
                        nc.vector.tensor_add(dst[rows, hs], t1[rows, :],
                                             t2[rows, :])
                        if is_kv:
                            nc.sync.dma_start(dst[0:64, hs],
                                              dst[64:128, hs])
                            if qh == 0:
                                nc.gpsimd.memset(v_aug[:].bitcast(f32),
                                                 1.0)
                            for tt in range(8 * qh, 8 * qh + 8):
                                ptr = psM.tile([128, 64], f32, tag="m",
                                               name=f"pt{qh}_{tt}")
                                nc.tensor.transpose(
                                    ptr[:],
                                    src[0:64, 128 * tt:128 * (tt + 1)],
                                    c_id[:])
                                nc.vector.tensor_copy(
                                    v_aug[:,
                                          (HD + 1) * tt:(HD + 1) * tt + HD],
                                    ptr[:])

            # consts for CDE (after AB's dma stream)
            nc.sync.dma_start(c_e2[:], e2)
            nc.sync.dma_start(c_mask[:], mask)
            for h in range(4):
                hr = slice(32 * h, 32 * h + 32)
                nc.sync.dma_start(c_wo0[hr, :], wo0[hr, :])
                nc.sync.dma_start(c_wo1[hr, :], wo1[hr, :])

            # ------- Fused phase C/D/E per q-chunk -------
            with tc.tile_pool(name="sbC", bufs=4) as sbC, \
                 tc.tile_pool(name="psS", bufs=2, space="PSUM") as psS, \
                 tc.tile_pool(name="psPV", bufs=2, space="PSUM") as psPV, \
                 tc.tile_pool(name="psO", bufs=2, space="PSUM") as psO:
                for qc in range(NQC):
                    qs = slice(512 * qc, 512 * qc + 512)
                    for hp, qr in ((0, qr0), (1, qr1)):
                        ppv_a = psPV.tile([65, 512], f32, tag="pv")
                        ppv_b = psPV.tile([65, 512], f32, tag="pv")
                        ntile = 4 * qc + 4
                        for t in range(ntile):
                            r = t - 4 * qc
                            off = max(0, r) * 128
                            qlo = 512 * qc + off
                            qlen = 512 * (qc + 1) - qlo
                            kc = slice(128 * t, 128 * (t + 1))
                            vs = slice((HD + 1) * t, (HD + 1) * t + HD + 1)
                            st = (t == 0)
                            sp = (t == ntile - 1)
                            ps_s = psS.tile([128, 1024], f32, tag="s")
                            nc.tensor.matmul(
                                ps_s[:, 0:qlen], krd[0:64, kc],
                                qr[0:64, qlo:qlo + qlen],
                                start=True, stop=True)
                            nc.tensor.matmul(
                                ps_s[:, 512:512 + qlen], krd[64:128, kc],
                                qr[64:128, qlo:qlo + qlen],
                                start=True, stop=True)
                            pt = sbC.tile([128, 1024], f32r, tag="pt")
                            if r >= 0:
                                nc.scalar.activation(
                                    pt[:, 0:512 + qlen],
                                    ps_s[:, 0:512 + qlen],
                                    AF.Exp, scale=SCALING)
                                nc.vector.tensor_mul(
                                    pt[:, 0:128], pt[:, 0:128], c_mask[:])
                                nc.vector.tensor_mul(
                                    pt[:, 512:640], pt[:, 512:640],
                                    c_mask[:])
                            else:
                                nc.scalar.activation(
                                    pt[:, 0:1024], ps_s[:, 0:1024],
                                    AF.Exp, scale=SCALING)
                            nc.tensor.matmul(
                                ppv_a[:, off:512], v_aug[:, vs],
                                pt[:, 0:qlen], start=st, stop=sp)
                            nc.tensor.matmul(
                                ppv_b[:, off:512], v_aug[:, vs],
                                pt[:, 512:512 + qlen], start=st, stop=sp)
                        for half, ppv in ((0, ppv_a), (1, ppv_b)):
                            stg = sbC.tile([65, 512], f32, tag="stg",
                                           bufs=3)
                            nc.vector.tensor_copy(stg[:], ppv[:])
                            nc.sync.dma_start(
                                attn_raw[hp][64 * half:64 * half + 64, qs],
                                stg[0:64, :])
                            nc.sync.dma_start(
                                l_sb[hp][half:half + 1, qs], stg[64:65, :])
                    # normalize this q-chunk: 1/l on DVE (keeps the
                    # ScalarE exp table resident through phase C)
                    for i in range(2):
                        rl = sbC.tile([2, 512], f32, tag="lnl", bufs=2,
                                      name=f"rl{i}")
                        nc.vector.reciprocal(rl[:], l_sb[i][:, qs])
                        rl_r = sbC.tile([2, 512], f32r, tag="rlr", bufs=2,
                                        name=f"rlr{i}")
                        nc.vector.tensor_copy(rl_r[:], rl[:])
                        pb = psO.tile([128, 512], f32, tag="o")
                        nc.tensor.matmul(pb[:], c_e2[:], rl_r[:],
                                         start=True, stop=True)
                        nc.vector.tensor_mul(
                            attn_raw[i][:, qs].bitcast(f32r),
                            attn_raw[i][:, qs], pb[:])
                    # o_proj rows + 256-row chunked reduce-scatter
                    for m in range(4 * qc, 4 * qc + 4):
                        ms = slice(128 * m, 128 * (m + 1))
                        ost = sbC.tile([128, S], rs_dt, tag="ost", bufs=2)
                        for n in range(4):
                            ns = slice(512 * n, 512 * n + 512)
                            po = psO.tile([128, 512], f32, tag="o")
                            nc.tensor.matmul(
                                po[:], attn_raw[0][:, ms].bitcast(f32r),
                                c_wo0[:, ns], start=True, stop=False)
                            nc.tensor.matmul(
                                po[:], attn_raw[1][:, ms].bitcast(f32r),
                                c_wo1[:, ns], start=False, stop=True)
                            nc.vector.tensor_copy(ost[:, ns], po[:])
                        nc.sync.dma_start(partial[ms, :], ost[:])
                        if m % 2 == 1:
                            ch = m // 2
                            nc.gpsimd.collective_compute(
                                "ReduceScatter",
                                mybir.AluOpType.add,
                                replica_groups=[list(range(N_CORES))],
                                ins=[partial[128 * (m - 1):128 * (m + 1),
                                             :].opt()],
                                outs=[rs_out[32 * ch:32 * ch + 32,
                                             :].opt()],
                            )
                            if RS_BF16:
                                stc = sbC.tile([32, S], rs_dt, tag="stc",
                                               bufs=2)
                                nc.sync.dma_start(
                                    stc[:], rs_out[32 * ch:32 * ch + 32, :])
                                stf = sbC.tile([32, S], f32, tag="stf",
                                               bufs=2)
                                nc.vector.tensor_copy(stf[:], stc[:])
                                nc.sync.dma_start(
                                    out_rs[32 * ch:32 * ch + 32, :],
                                    stf[:])
                            else:
                                nc.sync.dma_start(
                                    out_rs[32 * ch:32 * ch + 32, :],
                                    rs_out[32 * ch:32 * ch + 32,
                                           :].bitcast(f32))

    nc.compile()
    return nc


def _host_prep(hidden_states, position_ids, wq, wk, wv, wo, q_ln_w, k_ln_w):
    x = np.asarray(hidden_states, dtype=np.float32)[0]        # [S, HID]
    xT = np.ascontiguousarray(x.T)                            # [HID, S]
    pos = np.asarray(position_ids)[0].astype(np.float32)      # [S]
    inv = 1.0 / (ROPE_THETA ** (np.arange(0, HD, 2, dtype=np.float32) / HD))
    ang = pos[:, None] * inv[None, :]                         # [S, 32]
    emb = np.concatenate([ang, ang], axis=1)                  # [S, 64]
    cosT = np.cos(emb).T.astype(np.float32)                   # [64, S]
    sinT = np.sin(emb).T.astype(np.float32)
    ss = sinT.copy()
    ss[0:32] = -sinT[0:32]
    cos2 = np.tile(cosT, (2, 1))
    ss2 = np.tile(ss, (2, 1))

    e2 = np.zeros((2, 128), dtype=np.float32)
    e2[0, 0:64] = 1.0
    e2[1, 64:128] = 1.0
    ew_q = np.zeros((2, 128), dtype=np.float32)
    ew_q[0, 0:64] = q_ln_w
    ew_q[1, 64:128] = q_ln_w
    ew_k = np.zeros((2, 128), dtype=np.float32)
    ew_k[1, 64:128] = k_ln_w
    msk = (np.arange(128)[:, None] <= np.arange(128)[None, :]) \
        .astype(np.float32)
    ident = np.eye(64, dtype=np.float32)

    wq_ = np.asarray(wq, dtype=np.float32)
    wk_ = np.asarray(wk, dtype=np.float32)
    wv_ = np.asarray(wv, dtype=np.float32)
    wo_ = np.asarray(wo, dtype=np.float32)

    def pretile(w):  # [HID, 128] -> [128, HID] ktile-blocked
        return np.ascontiguousarray(
            w.reshape(NK, 128, 128).transpose(1, 0, 2).reshape(128, HID))

    in_maps = []
    for c in range(N_CORES):
        qcols = slice(256 * c, 256 * (c + 1))
        kvcols = slice(64 * c, 64 * (c + 1))
        wq_c = np.ascontiguousarray(wq_[:, qcols])
        wkv_c = np.concatenate([wv_[:, kvcols], wk_[:, kvcols]], axis=1)
        wo_c = np.ascontiguousarray(wo_[256 * c:256 * (c + 1), :])
        in_maps.append({
            "xT": xT,
            "wq0": pretile(wq_c[:, 0:128]),
            "wq1": pretile(wq_c[:, 128:256]),
            "wkv": pretile(wkv_c),
            "wo0": np.ascontiguousarray(wo_c[0:128, :]),
            "wo1": np.ascontiguousarray(wo_c[128:256, :]),
            "cos2": cos2,
            "ss2": ss2,
            "ew_q": ew_q,
            "ew_k": ew_k,
            "e2": e2,
            "e2t": np.ascontiguousarray(e2.T),
            "mask": msk,
            "ident": ident,
        })
    return in_maps


def kernel(hidden_states, position_ids, wq, wk, wv, wo, q_ln_w, k_ln_w):
    global _NC_CACHE, LAST_RESULTS
    if _NC_CACHE is None:
        _NC_CACHE = _build()
    nc = _NC_CACHE
    in_maps = _host_prep(hidden_states, position_ids, wq, wk, wv, wo,
                         q_ln_w, k_ln_w)
    res = bass_utils.run_bass_kernel_spmd(
        nc, in_maps, core_ids=list(range(N_CORES)))
    LAST_RESULTS = res
    out = np.empty((S, HID), dtype=np.float32)
    for c in range(N_CORES):
        o_c = res.results[c]["out_rs"]        # [256, 2048]
        for ch in range(8):
            out[256 * ch + 32 * c:256 * ch + 32 * c + 32, :] = \
                o_c[32 * ch:32 * ch + 32, :]
    return out.reshape(1, S, HID)



# revision 2
# speedup vs baseline: 1.0456x; 1.0456x over previous
"""GQA attention (B=1, S=2048, H=2048, 32 q-heads / 8 kv-heads, hd=64)
on 8 Trainium2 NeuronCores.

Sharding: tensor-parallel over heads. Core c owns q-heads 4c..4c+3 and
kv-head c: wq/wk/wv column shards, wo row shard; each core computes a
full [S, H] partial of the output projection; chunked bf16
ReduceScatters (256 output rows each) sum the partials; the host
scatters the per-core slices back together.

v2: all matmul operands bf16 (psum f32), software-pipelined per
512-column chunk c:
  A_c  projections (wq0/wq1/wkv stationary, xT chunk moving)
  B_c  per-head RMSNorm (ones-block stats matmul, Ln->Exp rstd) + RoPE
       (partition-shift sbuf DMA) + V transpose into v_aug
  C_c  causal attention for q-chunk c (scores^T -> exp -> P^T V_aug,
       split per kv-head for 1-bank psum tiles), 1/l via
       reciprocal_approx_fast, o_proj, ReduceScatter every 256 rows.
A_{c+1}'s matmuls are emitted as fillers interleaved into B_c/C_c so
TensorE never idles (keeps the HAM clock-gate at 2.4 GHz).
"""
import os
import sys

sys.path.insert(0, "/opt/trn_rl_repo")

import numpy as np  # noqa: E402
import ml_dtypes  # noqa: E402
import concourse.bacc as bacc  # noqa: E402
import concourse.mybir as mybir  # noqa: E402
import concourse.tile as tile  # noqa: E402
from concourse import bass_utils  # noqa: E402

f32 = mybir.dt.float32
bf16 = mybir.dt.bfloat16
AF = mybir.ActivationFunctionType
BF = ml_dtypes.bfloat16

N_CORES = 8
S = 2048
HID = 2048
HD = 64
ROPE_THETA = 10000.0
RMS_EPS = 1e-6
SCALING = HD ** -0.5              # 0.125
NK = HID // 128                   # 16 contraction tiles
NCH = 4                           # 512-col chunks

_NC_CACHE = None
LAST_RESULTS = None


def _build():
    nc = bacc.Bacc("TRN2", target_bir_lowering=False, debug=False,
                   num_devices=N_CORES)

    def din(name, shape, dt):
        return nc.dram_tensor(name, shape, dt, kind="ExternalInput").ap()

    xT = din("xT", [HID, S], bf16)
    # host-pretiled: row p, col block t = original rows 128t+p
    wq0 = din("wq0", [128, HID], bf16)
    wq1 = din("wq1", [128, HID], bf16)
    wkv = din("wkv", [128, HID], bf16)     # [wv | wk] columns pretiled
    wo0 = din("wo0", [128, S], bf16)
    wo1 = din("wo1", [128, S], bf16)
    cos2 = din("cos2", [128, S], bf16)
    ss2 = din("ss2", [128, S], bf16)
    ew_q = din("ew_q", [2, 128], bf16)
    ew_k = din("ew_k", [2, 128], bf16)
    e2 = din("e2", [2, 128], bf16)
    e2t = din("e2t", [128, 2], bf16)
    mask = din("mask", [128, 128], bf16)
    ident = din("ident", [64, 64], f32)

    out_rs = nc.dram_tensor("out_rs", [S // N_CORES, S], f32,
                            kind="ExternalOutput").ap()

    with tile.TileContext(nc) as tc:
        with tc.tile_pool(name="consts", bufs=1) as cp, \
             tc.tile_pool(name="dram", bufs=1, space="DRAM") as dp:
            c_wq0 = cp.tile([128, HID], bf16, tag="w")
            c_wq1 = cp.tile([128, HID], bf16, tag="w2")
            c_wkv = cp.tile([128, HID], bf16, tag="w3")
            c_wo0 = cp.tile([128, S], bf16, tag="w4")
            c_wo1 = cp.tile([128, S], bf16, tag="w5")
            c_cos = cp.tile([128, S], bf16, tag="c1")
            c_ss = cp.tile([128, S], bf16, tag="c2")
            c_ewq = cp.tile([2, 128], bf16, tag="c3")
            c_ewk = cp.tile([2, 128], bf16, tag="c4")
            c_e2 = cp.tile([2, 128], bf16, tag="c5")
            c_e2t = cp.tile([128, 2], bf16, tag="c5t")
            c_mask = cp.tile([128, 128], bf16, tag="c6")
            c_id = cp.tile([64, 64], f32, tag="c7")
            c_eps = cp.tile([2, 1], f32, tag="c8")

            # weights column-block first: matmul ktile t only needs
            # col block t//4, so the first matmul unblocks after ~3
            # small DMAs instead of the whole 1.5 MB
            for cb in range(4):
                cs = slice(512 * cb, 512 * cb + 512)
                nc.sync.dma_start(c_wq0[:, cs], wq0[:, cs])
                nc.sync.dma_start(c_wq1[:, cs], wq1[:, cs])
                nc.sync.dma_start(c_wkv[:, cs], wkv[:, cs])
            nc.vector.memset(c_eps[:], RMS_EPS)
            nc.sync.dma_start(c_e2t[:], e2t)
            nc.sync.dma_start(c_ewq[:], ew_q)
            nc.sync.dma_start(c_ewk[:], ew_k)
            nc.sync.dma_start(c_id[:], ident)
            nc.sync.dma_start(c_e2[:], e2)
            nc.sync.dma_start(c_mask[:], mask)
            nc.sync.dma_start(c_cos[:], cos2)
            nc.sync.dma_start(c_ss[:], ss2)
            for h in range(2):
                hr = slice(64 * h, 64 * h + 64)
                nc.sync.dma_start(c_wo0[hr, :], wo0[hr, :])
                nc.sync.dma_start(c_wo1[hr, :], wo1[hr, :])

            qkv = {
                "q0": cp.tile([128, S], f32, tag="q0", name="q0"),
                "q1": cp.tile([128, S], f32, tag="q1", name="q1"),
                "kv": cp.tile([128, S], f32, tag="kv", name="kv"),
            }
            qr0 = cp.tile([128, S], bf16, tag="qr0")
            qr1 = cp.tile([128, S], bf16, tag="qr1")
            krd = cp.tile([128, S], bf16, tag="krd")
            v_aug = cp.tile([128, NK * (HD + 1)], bf16, tag="vaug")

            partial = dp.tile([S, S], bf16)
            rs_out = dp.tile([S // N_CORES, S], bf16)

            with tc.tile_pool(name="xt", bufs=6) as xp, \
                 tc.tile_pool(name="sbB", bufs=2) as sbB, \
                 tc.tile_pool(name="sbC", bufs=2) as sbC, \
                 tc.tile_pool(name="psA", bufs=3, space="PSUM") as psA, \
                 tc.tile_pool(name="psS", bufs=2, space="PSUM") as psS, \
                 tc.tile_pool(name="psPV", bufs=2, space="PSUM") as psPV, \
                 tc.tile_pool(name="psB", bufs=1, space="PSUM") as psB:

                specs = [
                    ("kv", c_ewk, krd, True),
                    ("q0", c_ewq, qr0, False),
                    ("q1", c_ewq, qr1, False),
                ]

                def gen_A(c):
                    """Projection chunk c as a resumable generator:
                    one yield per ktile + one for the psum drains."""
                    cs = slice(512 * c, 512 * c + 512)
                    pa = [psA.tile([128, 512], f32, tag="pa",
                                   name=f"pa{c}_{j}") for j in range(3)]
                    for t in range(NK):
                        xt = xp.tile([128, 512], bf16, tag="xt",
                                     name=f"xt{c}_{t}")
                        nc.sync.dma_start(
                            xt[:], xT[128 * t:128 * t + 128, cs])
                        tc_ = slice(128 * t, 128 * (t + 1))
                        st = (t == 0)
                        sp = (t == NK - 1)
                        nc.tensor.matmul(pa[0][:], c_wq0[:, tc_], xt[:],
                                         start=st, stop=sp)
                        nc.tensor.matmul(pa[1][:], c_wq1[:, tc_], xt[:],
                                         start=st, stop=sp)
                        nc.tensor.matmul(pa[2][:], c_wkv[:, tc_], xt[:],
                                         start=st, stop=sp)
                        yield
                    nc.vector.tensor_copy(qkv["q0"][:, cs], pa[0][:])
                    yield
                    nc.scalar.copy(qkv["q1"][:, cs], pa[1][:])
                    nc.vector.tensor_copy(qkv["kv"][:, cs], pa[2][:])
                    yield

                filler = [None]

                def pop(n=1):
                    g = filler[0]
                    if g is None:
                        return
                    for _ in range(n):
                        try:
                            next(g)
                        except StopIteration:
                            filler[0] = None
                            return

                def drain():
                    pop(NK + 3)

                def emit_B(c):
                    cs = slice(512 * c, 512 * c + 512)
                    if c == 0:
                        nc.gpsimd.memset(v_aug[:], 1.0)
                    lnvs = []
                    for si, (key, ew, dst, is_kv) in enumerate(specs):
                        sq = sbB.tile([128, 512], bf16, tag="sq", bufs=2,
                                      name=f"sq{c}_{si}")
                        nc.vector.tensor_mul(sq[:], qkv[key][:, cs],
                                             qkv[key][:, cs])
                        pss = psB.tile([2, 512], f32, tag="m",
                                       name=f"ss{c}_{si}")
                        nc.tensor.matmul(pss[:], c_e2t[:], sq[:],
                                         start=True, stop=True)
                        lnv = sbB.tile([2, 512], f32, tag="lnv", bufs=3,
                                       name=f"lnv{c}{si}")
                        nc.scalar.activation(lnv[:], pss[:], AF.Ln,
                                             scale=1.0 / HD,
                                             bias=c_eps[:])
                        lnvs.append(lnv)
                        pop()
                    rstds = []
                    for si in range(3):
                        rr = sbB.tile([2, 512], bf16, tag="rstd", bufs=3,
                                      name=f"rr{c}{si}")
                        nc.scalar.activation(rr[:], lnvs[si][:],
                                             AF.Exp, scale=-0.5)
                        rstds.append(rr)
                    pop()
                    for si, (key, ew, dst, is_kv) in enumerate(specs):
                        rows = slice(64, 128) if is_kv else slice(0, 128)
                        pb = psB.tile([128, 512], f32, tag="m",
                                      name=f"pb{c}_{si}")
                        nc.tensor.matmul(pb[:], ew[:], rstds[si][:],
                                         start=True, stop=True)
                        nrm = sbB.tile([128, 512], bf16, tag="nrm",
                                       bufs=2, name=f"nrm{c}_{si}")
                        nc.vector.tensor_mul(nrm[rows, :],
                                             qkv[key][rows, cs],
                                             pb[rows, :])
                        # rope: rotate-half via partition-shift DMA
                        sh = sbB.tile([128, 512], bf16, tag="sh",
                                      bufs=2, name=f"sh{c}_{si}")
                        if is_kv:
                            nc.sync.dma_start(sh[64:96, :], nrm[96:128, :])
                            nc.sync.dma_start(sh[96:128, :], nrm[64:96, :])
                        else:
                            nc.sync.dma_start(sh[0:32, :], nrm[32:64, :])
                            nc.sync.dma_start(sh[32:64, :], nrm[0:32, :])
                            nc.sync.dma_start(sh[64:96, :], nrm[96:128, :])
                            nc.sync.dma_start(sh[96:128, :], nrm[64:96, :])
                        t2 = sbB.tile([128, 512], bf16, tag="t2",
                                      bufs=2, name=f"t2{c}_{si}")
                        nc.vector.tensor_mul(t2[rows, :], sh[rows, :],
                                             c_ss[rows, cs])
                        t1 = sbB.tile([128, 512], bf16, tag="t1",
                                      bufs=2, name=f"t1{c}_{si}")
                        nc.vector.tensor_mul(t1[rows, :], nrm[rows, :],
                                             c_cos[rows, cs])
                        nc.vector.tensor_add(dst[rows, cs], t1[rows, :],
                                             t2[rows, :])
                        pop()
                        if is_kv:
                            nc.sync.dma_start(dst[0:64, cs],
                                              dst[64:128, cs])
                            for tt in range(4 * c, 4 * c + 4):
                                ptr = psB.tile([128, 64], f32, tag="m",
                                               name=f"pt{c}_{tt}")
                                nc.tensor.transpose(
                                    ptr[:],
                                    qkv["kv"][0:64,
                                              128 * tt:128 * (tt + 1)],
                                    c_id[:])
                                nc.vector.tensor_copy(
                                    v_aug[:, (HD + 1) * tt:
                                          (HD + 1) * tt + HD],
                                    ptr[:])
                                pop()

                def emit_C(c):
                    qs = slice(512 * c, 512 * c + 512)
                    attnf = []
                    lsb = []
                    ntile = 4 * c + 4
                    for hp, qr in ((0, qr0), (1, qr1)):
                        ppv_a = psPV.tile([65, 512], f32, tag="pv",
                                          name=f"pva{c}_{hp}")
                        ppv_b = psPV.tile([65, 512], f32, tag="pv",
                                          name=f"pvb{c}_{hp}")
                        for t in range(ntile):
                            r = t - 4 * c
                            off = max(0, r) * 128
                            qlo = 512 * c + off
                            qlen = 512 * (c + 1) - qlo
                            kc = slice(128 * t, 128 * (t + 1))
                            vs = slice((HD + 1) * t,
                                       (HD + 1) * t + HD + 1)
                            st = (t == 0)
                            sp = (t == ntile - 1)
                            pop()
                            ps_a = psS.tile([128, 512], f32, tag="s",
                                            name=f"sa{c}{hp}{t}")
                            nc.tensor.matmul(
                                ps_a[:, 0:qlen], krd[0:64, kc],
                                qr[0:64, qlo:qlo + qlen],
                                start=True, stop=True)
                            ps_b = psS.tile([128, 512], f32, tag="s",
                                            name=f"sb{c}{hp}{t}")
                            nc.tensor.matmul(
                                ps_b[:, 0:qlen], krd[64:128, kc],
                                qr[64:128, qlo:qlo + qlen],
                                start=True, stop=True)
                            pt_a = sbC.tile([128, 512], bf16, tag="pt",
                                            bufs=4, name=f"pa{c}{hp}{t}")
                            nc.scalar.activation(pt_a[:, 0:qlen],
                                                 ps_a[:, 0:qlen],
                                                 AF.Exp, scale=SCALING)
                            if r >= 0:
                                nc.vector.tensor_mul(
                                    pt_a[:, 0:128], pt_a[:, 0:128],
                                    c_mask[:])
                            nc.tensor.matmul(
                                ppv_a[:, off:512], v_aug[:, vs],
                                pt_a[:, 0:qlen], start=st, stop=sp)
                            pt_b = sbC.tile([128, 512], bf16, tag="pt",
                                            bufs=4, name=f"pb{c}{hp}{t}")
                            nc.scalar.activation(pt_b[:, 0:qlen],
                                                 ps_b[:, 0:qlen],
                                                 AF.Exp, scale=SCALING)
                            if r >= 0:
                                nc.vector.tensor_mul(
                                    pt_b[:, 0:128], pt_b[:, 0:128],
                                    c_mask[:])
                            nc.tensor.matmul(
                                ppv_b[:, off:512], v_aug[:, vs],
                                pt_b[:, 0:qlen], start=st, stop=sp)
                        # drain ppv -> attn rows + l row
                        af = sbC.tile([128, 512], f32, tag="attnf",
                                      bufs=2, name=f"af{c}_{hp}")
                        ls = sbC.tile([2, 512], f32, tag="l", bufs=2,
                                      name=f"ls{c}_{hp}")
                        for half, ppv in ((0, ppv_a), (1, ppv_b)):
                            stg = sbC.tile([65, 512], f32, tag="stg",
                                           bufs=2, name=f"st{c}{hp}{half}")
                            if half == 0:
                                nc.vector.tensor_copy(stg[:], ppv[:])
                            else:
                                nc.scalar.copy(stg[:], ppv[:])
                            nc.sync.dma_start(
                                af[64 * half:64 * half + 64, :],
                                stg[0:64, :])
                            nc.sync.dma_start(
                                ls[half:half + 1, :], stg[64:65, :])
                            pop()
                        attnf.append(af)
                        lsb.append(ls)
                    # normalize by 1/l, cast to bf16 for o_proj
                    attnb = []
                    for i in range(2):
                        rl = sbC.tile([2, 512], f32, tag="rl", bufs=2,
                                      name=f"rl{c}{i}")
                        nc.vector.reciprocal_approx_fast(rl[:], lsb[i][:])
                        rlb = sbC.tile([2, 512], bf16, tag="rlb", bufs=2,
                                       name=f"rlb{c}{i}")
                        nc.vector.tensor_copy(rlb[:], rl[:])
                        pb = psPV.tile([128, 512], f32, tag="pv",
                                       name=f"plb{c}{i}")
                        nc.tensor.matmul(pb[:], c_e2[:], rlb[:],
                                         start=True, stop=True)
                        ab = sbC.tile([128, 512], bf16, tag="attnb",
                                      bufs=2, name=f"ab{c}_{i}")
                        nc.vector.tensor_mul(ab[:], attnf[i][:], pb[:])
                        attnb.append(ab)
                        pop()
                    # o_proj rows + 256-row chunked reduce-scatter
                    for ml in range(4):
                        m = 4 * c + ml
                        mls = slice(128 * ml, 128 * ml + 128)
                        ost = sbC.tile([128, S], bf16, tag="ost", bufs=2,
                                       name=f"ost{c}_{ml}")
                        for n in range(4):
                            ns = slice(512 * n, 512 * n + 512)
                            po = psPV.tile([128, 512], f32, tag="pv",
                                           name=f"po{c}{ml}{n}")
                            nc.tensor.matmul(
                                po[:], attnb[0][:, mls], c_wo0[:, ns],
                                start=True, stop=False)
                            nc.tensor.matmul(
                                po[:], attnb[1][:, mls], c_wo1[:, ns],
                                start=False, stop=True)
                            if n % 2 == 0:
                                nc.vector.tensor_copy(ost[:, ns], po[:])
                            else:
                                nc.scalar.copy(ost[:, ns], po[:])
                            pop()
                        nc.sync.dma_start(
                            partial[128 * m:128 * (m + 1), :], ost[:])
                        if ml % 2 == 1:
                            ch = m // 2
                            nc.gpsimd.collective_compute(
                                "ReduceScatter",
                                mybir.AluOpType.add,
                                replica_groups=[list(range(N_CORES))],
                                ins=[partial[128 * (m - 1):128 * (m + 1),
                                             :].opt()],
                                outs=[rs_out[32 * ch:32 * ch + 32,
                                             :].opt()],
                            )
                            stc = sbC.tile([32, S], bf16, tag="stc",
                                           bufs=2, name=f"sc{c}_{ml}")
                            nc.sync.dma_start(
                                stc[:], rs_out[32 * ch:32 * ch + 32, :])
                            stf = sbC.tile([32, S], f32, tag="stf",
                                           bufs=2, name=f"sf{c}_{ml}")
                            nc.gpsimd.tensor_copy(stf[:], stc[:])
                            nc.sync.dma_start(
                                out_rs[32 * ch:32 * ch + 32, :], stf[:])

                # A_0 runs undeferred; each later A chunk drips into
                # the previous B/C as fillers
                for _ in gen_A(0):
                    pass
                for c in range(NCH):
                    if c + 1 < NCH:
                        filler[0] = gen_A(c + 1)
                    emit_B(c)
                    emit_C(c)
                    drain()

    nc.compile()
    return nc


def _host_prep(hidden_states, position_ids, wq, wk, wv, wo, q_ln_w, k_ln_w):
    x = np.asarray(hidden_states, dtype=np.float32)[0]        # [S, HID]
    xT = np.ascontiguousarray(x.T).astype(BF)                 # [HID, S]
    pos = np.asarray(position_ids)[0].astype(np.float32)      # [S]
    inv = 1.0 / (ROPE_THETA ** (np.arange(0, HD, 2, dtype=np.float32) / HD))
    ang = pos[:, None] * inv[None, :]                         # [S, 32]
    emb = np.concatenate([ang, ang], axis=1)                  # [S, 64]
    cosT = np.cos(emb).T.astype(np.float32)                   # [64, S]
    sinT = np.sin(emb).T.astype(np.float32)
    ss = sinT.copy()
    ss[0:32] = -sinT[0:32]
    cos2 = np.tile(cosT, (2, 1)).astype(BF)
    ss2 = np.tile(ss, (2, 1)).astype(BF)

    e2 = np.zeros((2, 128), dtype=np.float32)
    e2[0, 0:64] = 1.0
    e2[1, 64:128] = 1.0
    ew_q = np.zeros((2, 128), dtype=np.float32)
    ew_q[0, 0:64] = q_ln_w
    ew_q[1, 64:128] = q_ln_w
    ew_k = np.zeros((2, 128), dtype=np.float32)
    ew_k[1, 64:128] = k_ln_w
    msk = (np.arange(128)[:, None] <= np.arange(128)[None, :]) \
        .astype(BF)
    ident = np.eye(64, dtype=np.float32)

    wq_ = np.asarray(wq, dtype=np.float32)
    wk_ = np.asarray(wk, dtype=np.float32)
    wv_ = np.asarray(wv, dtype=np.float32)
    wo_ = np.asarray(wo, dtype=np.float32)

    def pretile(w):  # [HID, 128] -> [128, HID] ktile-blocked
        return np.ascontiguousarray(
            w.reshape(NK, 128, 128).transpose(1, 0, 2).reshape(128, HID)
        ).astype(BF)

    in_maps = []
    for c in range(N_CORES):
        qcols = slice(256 * c, 256 * (c + 1))
        kvcols = slice(64 * c, 64 * (c + 1))
        wq_c = np.ascontiguousarray(wq_[:, qcols])
        wkv_c = np.concatenate([wv_[:, kvcols], wk_[:, kvcols]], axis=1)
        wo_c = np.ascontiguousarray(wo_[256 * c:256 * (c + 1), :])
        in_maps.append({
            "xT": xT,
            "wq0": pretile(wq_c[:, 0:128]),
            "wq1": pretile(wq_c[:, 128:256]),
            "wkv": pretile(wkv_c),
            "wo0": np.ascontiguousarray(wo_c[0:128, :]).astype(BF),
            "wo1": np.ascontiguousarray(wo_c[128:256, :]).astype(BF),
            "cos2": cos2,
            "ss2": ss2,
            "ew_q": ew_q.astype(BF),
            "ew_k": ew_k.astype(BF),
            "e2": e2.astype(BF),
            "e2t": np.ascontiguousarray(e2.T).astype(BF),
            "mask": msk,
            "ident": ident,
        })
    return in_maps


def kernel(hidden_states, position_ids, wq, wk, wv, wo, q_ln_w, k_ln_w):
    global _NC_CACHE, LAST_RESULTS
    if _NC_CACHE is None:
        _NC_CACHE = _build()
    nc = _NC_CACHE
    in_maps = _host_prep(hidden_states, position_ids, wq, wk, wv, wo,
                         q_ln_w, k_ln_w)
    res = bass_utils.run_bass_kernel_spmd(
        nc, in_maps, core_ids=list(range(N_CORES)))
    LAST_RESULTS = res
    out = np.empty((S, HID), dtype=np.float32)
    for c in range(N_CORES):
        o_c = res.results[c]["out_rs"]        # [256, 2048]
        for ch in range(8):
            out[256 * ch + 32 * c:256 * ch + 32 * c + 32, :] = \
                o_c[32 * ch:32 * ch + 32, :]
    return out.reshape(1, S, HID)


# revision 3
# speedup vs baseline: 1.0773x; 1.0304x over previous
"""GQA attention (B=1, S=2048, H=2048, 32 q-heads / 8 kv-heads, hd=64)
on 8 Trainium2 NeuronCores.

Sharding: tensor-parallel over heads. Core c owns q-heads 4c..4c+3 and
kv-head c: wq/wk/wv column shards, wo row shard; each core computes a
full [S, H] partial of the output projection; chunked bf16
ReduceScatters (256 output rows each) sum the partials; the host
scatters the per-core slices back together.

v3: all matmul operands bf16 (psum f32), software-pipelined per
512-column chunk c:
  A_c  projections (wq0/wq1/wkv stationary, persistent xT tiles moving)
  B_c  per-head RMSNorm: the 3 stats matmuls write one [66,512] psum
       tile at partition offsets 0/32/64, so a single Ln and a single
       Exp cover all of q0/q1/kv (2 act-table loads per chunk) + RoPE
       (partition-shift sbuf DMA issued from GpSimd) + V transpose
  C_c  causal attention for q-chunk c (scores^T -> exp -> P^T V_aug,
       split per kv-head for 1-bank psum tiles), 1/l via
       reciprocal_approx_fast, o_proj, ReduceScatter every 256 rows.
A_{c+1}'s matmuls are emitted as fillers interleaved into B_c/C_c so
TensorE never idles (keeps the HAM clock-gate at 2.4 GHz).
"""
import os
import sys

sys.path.insert(0, "/opt/trn_rl_repo")

import numpy as np  # noqa: E402
import ml_dtypes  # noqa: E402
import concourse.bacc as bacc  # noqa: E402
import concourse.mybir as mybir  # noqa: E402
import concourse.tile as tile  # noqa: E402
from concourse import bass_utils  # noqa: E402

f32 = mybir.dt.float32
bf16 = mybir.dt.bfloat16
AF = mybir.ActivationFunctionType
BF = ml_dtypes.bfloat16

N_CORES = 8
S = 2048
HID = 2048
HD = 64
ROPE_THETA = 10000.0
RMS_EPS = 1e-6
SCALING = HD ** -0.5              # 0.125
NK = HID // 128                   # 16 contraction tiles
NCH = 4                           # 512-col chunks

_NC_CACHE = None
LAST_RESULTS = None


def _build():
    nc = bacc.Bacc("TRN2", target_bir_lowering=False, debug=False,
                   num_devices=N_CORES)

    def din(name, shape, dt):
        return nc.dram_tensor(name, shape, dt, kind="ExternalInput").ap()

    xT = din("xT", [HID, S], bf16)
    # host-pretiled: row p, col block t = original rows 128t+p
    wq0 = din("wq0", [128, HID], bf16)
    wq1 = din("wq1", [128, HID], bf16)
    wkv = din("wkv", [128, HID], bf16)     # [wv | wk] columns pretiled
    wo0 = din("wo0", [128, S], bf16)
    wo1 = din("wo1", [128, S], bf16)
    cos2 = din("cos2", [128, S], bf16)
    ss2 = din("ss2", [128, S], bf16)
    ew_all = din("ew_all", [66, 128], bf16)
    e2 = din("e2", [2, 128], bf16)
    e2t = din("e2t", [128, 66], bf16)
    mask = din("mask", [128, 128], bf16)
    ident = din("ident", [64, 64], f32)

    out_rs = nc.dram_tensor("out_rs", [S // N_CORES, S], f32,
                            kind="ExternalOutput").ap()

    with tile.TileContext(nc) as tc:
        with tc.tile_pool(name="consts", bufs=1) as cp, \
             tc.tile_pool(name="dram", bufs=1, space="DRAM") as dp:
            c_wq0 = cp.tile([128, HID], bf16, tag="w")
            c_wq1 = cp.tile([128, HID], bf16, tag="w2")
            c_wkv = cp.tile([128, HID], bf16, tag="w3")
            c_wo0 = cp.tile([128, S], bf16, tag="w4")
            c_wo1 = cp.tile([128, S], bf16, tag="w5")
            c_cos = cp.tile([128, S], bf16, tag="c1")
            c_ss = cp.tile([128, S], bf16, tag="c2")
            c_ew = cp.tile([66, 128], bf16, tag="c3")
            c_e2 = cp.tile([2, 128], bf16, tag="c5")
            c_e2t = cp.tile([128, 66], bf16, tag="c5t")
            c_mask = cp.tile([128, 128], bf16, tag="c6")
            c_id = cp.tile([64, 64], f32, tag="c7")
            c_eps = cp.tile([66, 1], f32, tag="c8")
            xts = [cp.tile([128, S], bf16, tag=f"x{t}", name=f"xts{t}")
                   for t in range(NK)]

            # weights column-block first: matmul ktile t only needs
            # col block t//4, so the first matmul unblocks after ~3
            # small DMAs instead of the whole 1.5 MB
            for cb in range(4):
                cs = slice(512 * cb, 512 * cb + 512)
                nc.sync.dma_start(c_wq0[:, cs], wq0[:, cs])
                nc.sync.dma_start(c_wq1[:, cs], wq1[:, cs])
                nc.sync.dma_start(c_wkv[:, cs], wkv[:, cs])
            nc.vector.memset(c_eps[:], RMS_EPS)
            nc.sync.dma_start(c_e2t[:], e2t)
            nc.sync.dma_start(c_ew[:], ew_all)
            nc.sync.dma_start(c_id[:], ident)
            nc.sync.dma_start(c_e2[:], e2)
            nc.sync.dma_start(c_mask[:], mask)
            # x row-blocks, first halves first (chunks 0/1 only touch
            # cols 0:1024); contiguous 2KB partition lines
            for t in range(NK):
                nc.sync.dma_start(xts[t][:, 0:1024],
                                  xT[128 * t:128 * t + 128, 0:1024])
                if t == 3:
                    nc.sync.dma_start(c_cos[:], cos2)
                    nc.sync.dma_start(c_ss[:], ss2)
            for h in range(2):
                hr = slice(64 * h, 64 * h + 64)
                nc.sync.dma_start(c_wo0[hr, :], wo0[hr, :])
                nc.sync.dma_start(c_wo1[hr, :], wo1[hr, :])
            for t in range(NK):
                nc.sync.dma_start(xts[t][:, 1024:2048],
                                  xT[128 * t:128 * t + 128, 1024:2048])

            qkv = {
                "q0": cp.tile([128, S], f32, tag="q0", name="q0"),
                "q1": cp.tile([128, S], f32, tag="q1", name="q1"),
                "kv": cp.tile([128, S], f32, tag="kv", name="kv"),
            }
            qr0 = cp.tile([128, S], bf16, tag="qr0")
            qr1 = cp.tile([128, S], bf16, tag="qr1")
            krd = cp.tile([128, S], bf16, tag="krd")
            v_aug = cp.tile([128, NK * (HD + 1)], bf16, tag="vaug")

            partial = dp.tile([S, S], bf16)
            rs_out = dp.tile([S // N_CORES, S], bf16)

            with tc.tile_pool(name="sbB", bufs=2) as sbB, \
                 tc.tile_pool(name="sbC", bufs=2) as sbC, \
                 tc.tile_pool(name="psA", bufs=3, space="PSUM") as psA, \
                 tc.tile_pool(name="psS", bufs=2, space="PSUM") as psS, \
                 tc.tile_pool(name="psPV", bufs=2, space="PSUM") as psPV, \
                 tc.tile_pool(name="psB", bufs=1, space="PSUM") as psB:

                # si 0 = kv (write rows 64:128 -> partition base 0),
                # si 1 = q0, si 2 = q1
                specs = [
                    ("kv", krd, True),
                    ("q0", qr0, False),
                    ("q1", qr1, False),
                ]

                def gen_A(c):
                    """Projection chunk c as a resumable generator:
                    one yield per ktile + two for the psum drains."""
                    cs = slice(512 * c, 512 * c + 512)
                    pa = [psA.tile([128, 512], f32, tag="pa",
                                   name=f"pa{c}_{j}") for j in range(3)]
                    for t in range(NK):
                        tc_ = slice(128 * t, 128 * (t + 1))
                        st = (t == 0)
                        sp = (t == NK - 1)
                        nc.tensor.matmul(pa[0][:], c_wq0[:, tc_],
                                         xts[t][:, cs], start=st, stop=sp)
                        nc.tensor.matmul(pa[1][:], c_wq1[:, tc_],
                                         xts[t][:, cs], start=st, stop=sp)
                        nc.tensor.matmul(pa[2][:], c_wkv[:, tc_],
                                         xts[t][:, cs], start=st, stop=sp)
                        yield
                    nc.vector.tensor_copy(qkv["q0"][:, cs], pa[0][:])
                    yield
                    nc.scalar.copy(qkv["q1"][:, cs], pa[1][:])
                    nc.vector.tensor_copy(qkv["kv"][:, cs], pa[2][:])
                    yield

                filler = [None]

                def pop(n=1):
                    g = filler[0]
                    if g is None:
                        return
                    for _ in range(n):
                        try:
                            next(g)
                        except StopIteration:
                            filler[0] = None
                            return

                def drain():
                    pop(NK + 3)

                def emit_B(c):
                    cs = slice(512 * c, 512 * c + 512)
                    if c == 0:
                        nc.gpsimd.memset(v_aug[:], 1.0)
                    # fused stats: one [66,512] psum tile, partition
                    # base 32*si per spec -> single Ln + single Exp
                    pss = psB.tile([66, 512], f32, tag="m",
                                   name=f"ss{c}")
                    for si, (key, dst, is_kv) in enumerate(specs):
                        sq = sbB.tile([128, 512], bf16, tag="sq", bufs=2,
                                      name=f"sq{c}_{si}")
                        nc.vector.tensor_mul(sq[:], qkv[key][:, cs],
                                             qkv[key][:, cs])
                        nc.tensor.matmul(pss[32 * si:32 * si + 2, :],
                                         c_e2t[:, 32 * si:32 * si + 2],
                                         sq[:], start=True, stop=True)
                        pop()
                    lnv = sbB.tile([66, 512], f32, tag="lnv", bufs=2,
                                   name=f"lnv{c}")
                    nc.scalar.activation(lnv[:], pss[:], AF.Ln,
                                         scale=1.0 / HD, bias=c_eps[:])
                    rstd = sbB.tile([66, 512], bf16, tag="rstd", bufs=2,
                                    name=f"rr{c}")
                    nc.scalar.activation(rstd[:], lnv[:],
                                         AF.Exp, scale=-0.5)
                    pop()
                    for si, (key, dst, is_kv) in enumerate(specs):
                        rows = slice(64, 128) if is_kv else slice(0, 128)
                        ps_ = slice(32 * si, 32 * si + 2)
                        pb = psB.tile([128, 512], f32, tag="m",
                                      name=f"pb{c}_{si}")
                        nc.tensor.matmul(pb[:], c_ew[ps_, :],
                                         rstd[ps_, :],
                                         start=True, stop=True)
                        nrm = sbB.tile([128, 512], bf16, tag="nrm",
                                       bufs=2, name=f"nrm{c}_{si}")
                        nc.vector.tensor_mul(nrm[rows, :],
                                             qkv[key][rows, cs],
                                             pb[rows, :])
                        # rope: rotate-half via partition-shift DMA
                        sh = sbB.tile([128, 512], bf16, tag="sh",
                                      bufs=2, name=f"sh{c}_{si}")
                        if is_kv:
                            nc.gpsimd.dma_start(sh[64:96, :],
                                                nrm[96:128, :])
                            nc.gpsimd.dma_start(sh[96:128, :],
                                                nrm[64:96, :])
                        else:
                            nc.gpsimd.dma_start(sh[0:32, :],
                                                nrm[32:64, :])
                            nc.gpsimd.dma_start(sh[32:64, :],
                                                nrm[0:32, :])
                            nc.gpsimd.dma_start(sh[64:96, :],
                                                nrm[96:128, :])
                            nc.gpsimd.dma_start(sh[96:128, :],
                                                nrm[64:96, :])
                        t2 = sbB.tile([128, 512], bf16, tag="t2",
                                      bufs=2, name=f"t2{c}_{si}")
                        nc.vector.tensor_mul(t2[rows, :], sh[rows, :],
                                             c_ss[rows, cs])
                        t1 = sbB.tile([128, 512], bf16, tag="t1",
                                      bufs=2, name=f"t1{c}_{si}")
                        nc.vector.tensor_mul(t1[rows, :], nrm[rows, :],
                                             c_cos[rows, cs])
                        nc.vector.tensor_add(dst[rows, cs], t1[rows, :],
                                             t2[rows, :])
                        pop()
                        if is_kv:
                            nc.gpsimd.dma_start(dst[0:64, cs],
                                                dst[64:128, cs])
                            for tt in range(4 * c, 4 * c + 4):
                                ptr = psB.tile([128, 64], f32, tag="m",
                                               name=f"pt{c}_{tt}")
                                nc.tensor.transpose(
                                    ptr[:],
                                    qkv["kv"][0:64,
                                              128 * tt:128 * (tt + 1)],
                                    c_id[:])
                                nc.vector.tensor_copy(
                                    v_aug[:, (HD + 1) * tt:
                                          (HD + 1) * tt + HD],
                                    ptr[:])
                                pop()

                def emit_C(c):
                    attnf = []
                    lsb = []
                    ntile = 4 * c + 4
                    for hp, qr in ((0, qr0), (1, qr1)):
                        ppv_a = psPV.tile([65, 512], f32, tag="pv",
                                          name=f"pva{c}_{hp}")
                        ppv_b = psPV.tile([65, 512], f32, tag="pv",
                                          name=f"pvb{c}_{hp}")
                        for t in range(ntile):
                            r = t - 4 * c
                            off = max(0, r) * 128
                            qlo = 512 * c + off
                            qlen = 512 * (c + 1) - qlo
                            kc = slice(128 * t, 128 * (t + 1))
                            vs = slice((HD + 1) * t,
                                       (HD + 1) * t + HD + 1)
                            st = (t == 0)
                            sp = (t == ntile - 1)
                            pop()
                            ps_a = psS.tile([128, 512], f32, tag="s",
                                            name=f"sa{c}{hp}{t}")
                            nc.tensor.matmul(
                                ps_a[:, 0:qlen], krd[0:64, kc],
                                qr[0:64, qlo:qlo + qlen],
                                start=True, stop=True)
                            ps_b = psS.tile([128, 512], f32, tag="s",
                                            name=f"sb{c}{hp}{t}")
                            nc.tensor.matmul(
                                ps_b[:, 0:qlen], krd[64:128, kc],
                                qr[64:128, qlo:qlo + qlen],
                                start=True, stop=True)
                            pt_a = sbC.tile([128, 512], bf16, tag="pt",
                                            bufs=4, name=f"pa{c}{hp}{t}")
                            nc.scalar.activation(pt_a[:, 0:qlen],
                                                 ps_a[:, 0:qlen],
                                                 AF.Exp, scale=SCALING)
                            if r >= 0:
                                nc.vector.tensor_mul(
                                    pt_a[:, 0:128], pt_a[:, 0:128],
                                    c_mask[:])
                            nc.tensor.matmul(
                                ppv_a[:, off:512], v_aug[:, vs],
                                pt_a[:, 0:qlen], start=st, stop=sp)
                            pt_b = sbC.tile([128, 512], bf16, tag="pt",
                                            bufs=4, name=f"pb{c}{hp}{t}")
                            nc.scalar.activation(pt_b[:, 0:qlen],
                                                 ps_b[:, 0:qlen],
                                                 AF.Exp, scale=SCALING)
                            if r >= 0:
                                nc.vector.tensor_mul(
                                    pt_b[:, 0:128], pt_b[:, 0:128],
                                    c_mask[:])
                            nc.tensor.matmul(
                                ppv_b[:, off:512], v_aug[:, vs],
                                pt_b[:, 0:qlen], start=st, stop=sp)
                        # drain ppv -> attn rows + l row
                        af = sbC.tile([128, 512], f32, tag="attnf",
                                      bufs=2, name=f"af{c}_{hp}")
                        ls = sbC.tile([2, 512], f32, tag="l", bufs=2,
                                      name=f"ls{c}_{hp}")
                        for half, ppv in ((0, ppv_a), (1, ppv_b)):
                            stg = sbC.tile([65, 512], f32, tag="stg",
                                           bufs=2, name=f"st{c}{hp}{half}")
                            if half == 0:
                                nc.vector.tensor_copy(stg[:], ppv[:])
                            else:
                                nc.scalar.copy(stg[:], ppv[:])
                            nc.gpsimd.dma_start(
                                af[64 * half:64 * half + 64, :],
                                stg[0:64, :])
                            nc.gpsimd.dma_start(
                                ls[half:half + 1, :], stg[64:65, :])
                            pop()
                        attnf.append(af)
                        lsb.append(ls)
                    # normalize by 1/l, cast to bf16 for o_proj
                    attnb = []
                    for i in range(2):
                        rl = sbC.tile([2, 512], f32, tag="rl", bufs=2,
                                      name=f"rl{c}{i}")
                        nc.vector.reciprocal_approx_fast(rl[:], lsb[i][:])
                        rlb = sbC.tile([2, 512], bf16, tag="rlb", bufs=2,
                                       name=f"rlb{c}{i}")
                        nc.vector.tensor_copy(rlb[:], rl[:])
                        pb = psPV.tile([128, 512], f32, tag="pv",
                                       name=f"plb{c}{i}")
                        nc.tensor.matmul(pb[:], c_e2[:], rlb[:],
                                         start=True, stop=True)
                        ab = sbC.tile([128, 512], bf16, tag="attnb",
                                      bufs=2, name=f"ab{c}_{i}")
                        nc.vector.tensor_mul(ab[:], attnf[i][:], pb[:])
                        attnb.append(ab)
                        pop()
                    # o_proj rows + 256-row chunked reduce-scatter
                    for ml in range(4):
                        m = 4 * c + ml
                        mls = slice(128 * ml, 128 * ml + 128)
                        ost = sbC.tile([128, S], bf16, tag="ost", bufs=2,
                                       name=f"ost{c}_{ml}")
                        for n in range(4):
                            ns = slice(512 * n, 512 * n + 512)
                            po = psPV.tile([128, 512], f32, tag="pv",
                                           name=f"po{c}{ml}{n}")
                            nc.tensor.matmul(
                                po[:], attnb[0][:, mls], c_wo0[:, ns],
                                start=True, stop=False)
                            nc.tensor.matmul(
                                po[:], attnb[1][:, mls], c_wo1[:, ns],
                                start=False, stop=True)
                            if n % 2 == 0:
                                nc.vector.tensor_copy(ost[:, ns], po[:])
                            else:
                                nc.scalar.copy(ost[:, ns], po[:])
                            pop()
                        nc.sync.dma_start(
                            partial[128 * m:128 * (m + 1), :], ost[:])
                        if ml % 2 == 1:
                            ch = m // 2
                            nc.gpsimd.collective_compute(
                                "ReduceScatter",
                                mybir.AluOpType.add,
                                replica_groups=[list(range(N_CORES))],
                                ins=[partial[128 * (m - 1):128 * (m + 1),
                                             :].opt()],
                                outs=[rs_out[32 * ch:32 * ch + 32,
                                             :].opt()],
                            )
                            stc = sbC.tile([32, S], bf16, tag="stc",
                                           bufs=2, name=f"sc{c}_{ml}")
                            nc.gpsimd.dma_start(
                                stc[:], rs_out[32 * ch:32 * ch + 32, :])
                            stf = sbC.tile([32, S], f32, tag="stf",
                                           bufs=2, name=f"sf{c}_{ml}")
                            nc.vector.tensor_copy(stf[:], stc[:])
                            nc.gpsimd.dma_start(
                                out_rs[32 * ch:32 * ch + 32, :], stf[:])

                # A_0 runs undeferred; each later A chunk drips into
                # the previous B/C as fillers
                for _ in gen_A(0):
                    pass
                for c in range(NCH):
                    if c + 1 < NCH:
                        filler[0] = gen_A(c + 1)
                    emit_B(c)
                    emit_C(c)
                    drain()

    nc.compile()
    return nc


def _host_prep(hidden_states, position_ids, wq, wk, wv, wo, q_ln_w, k_ln_w):
    x = np.asarray(hidden_states, dtype=np.float32)[0]        # [S, HID]
    xT = np.ascontiguousarray(x.T).astype(BF)                 # [HID, S]
    pos = np.asarray(position_ids)[0].astype(np.float32)      # [S]
    inv = 1.0 / (ROPE_THETA ** (np.arange(0, HD, 2, dtype=np.float32) / HD))
    ang = pos[:, None] * inv[None, :]                         # [S, 32]
    emb = np.concatenate([ang, ang], axis=1)                  # [S, 64]
    cosT = np.cos(emb).T.astype(np.float32)                   # [64, S]
    sinT = np.sin(emb).T.astype(np.float32)
    ss = sinT.copy()
    ss[0:32] = -sinT[0:32]
    cos2 = np.tile(cosT, (2, 1)).astype(BF)
    ss2 = np.tile(ss, (2, 1)).astype(BF)

    e2 = np.zeros((2, 128), dtype=np.float32)
    e2[0, 0:64] = 1.0
    e2[1, 64:128] = 1.0
    # fused stats layout: si 0 (kv) at partitions 0:2, si 1 (q0) at
    # 32:34, si 2 (q1) at 64:66; row pairs select [rows 0:64, 64:128]
    ew_all = np.zeros((66, 128), dtype=np.float32)
    ew_all[1, 64:128] = k_ln_w          # kv: k rows live at 64:128
    ew_all[32, 0:64] = q_ln_w
    ew_all[33, 64:128] = q_ln_w
    ew_all[64, 0:64] = q_ln_w
    ew_all[65, 64:128] = q_ln_w
    e2t = np.zeros((128, 66), dtype=np.float32)
    e2t[0:64, 0] = 1.0
    e2t[64:128, 1] = 1.0
    e2t[0:64, 32] = 1.0
    e2t[64:128, 33] = 1.0
    e2t[0:64, 64] = 1.0
    e2t[64:128, 65] = 1.0
    msk = (np.arange(128)[:, None] <= np.arange(128)[None, :]) \
        .astype(BF)
    ident = np.eye(64, dtype=np.float32)

    wq_ = np.asarray(wq, dtype=np.float32)
    wk_ = np.asarray(wk, dtype=np.float32)
    wv_ = np.asarray(wv, dtype=np.float32)
    wo_ = np.asarray(wo, dtype=np.float32)

    def pretile(w):  # [HID, 128] -> [128, HID] ktile-blocked
        return np.ascontiguousarray(
            w.reshape(NK, 128, 128).transpose(1, 0, 2).reshape(128, HID)
        ).astype(BF)

    in_maps = []
    for c in range(N_CORES):
        qcols = slice(256 * c, 256 * (c + 1))
        kvcols = slice(64 * c, 64 * (c + 1))
        wq_c = np.ascontiguousarray(wq_[:, qcols])
        wkv_c = np.concatenate([wv_[:, kvcols], wk_[:, kvcols]], axis=1)
        wo_c = np.ascontiguousarray(wo_[256 * c:256 * (c + 1), :])
        in_maps.append({
            "xT": xT,
            "wq0": pretile(wq_c[:, 0:128]),
            "wq1": pretile(wq_c[:, 128:256]),
            "wkv": pretile(wkv_c),
            "wo0": np.ascontiguousarray(wo_c[0:128, :]).astype(BF),
            "wo1": np.ascontiguousarray(wo_c[128:256, :]).astype(BF),
            "cos2": cos2,
            "ss2": ss2,
            "ew_all": ew_all.astype(BF),
            "e2": e2.astype(BF),
            "e2t": np.ascontiguousarray(e2t).astype(BF),
            "mask": msk,
            "ident": ident,
        })
    return in_maps


def kernel(hidden_states, position_ids, wq, wk, wv, wo, q_ln_w, k_ln_w):
    global _NC_CACHE, LAST_RESULTS
    if _NC_CACHE is None:
        _NC_CACHE = _build()
    nc = _NC_CACHE
    in_maps = _host_prep(hidden_states, position_ids, wq, wk, wv, wo,
                         q_ln_w, k_ln_w)
    res = bass_utils.run_bass_kernel_spmd(
        nc, in_maps, core_ids=list(range(N_CORES)))
    LAST_RESULTS = res
    out = np.empty((S, HID), dtype=np.float32)
    for c in range(N_CORES):
        o_c = res.results[c]["out_rs"]        # [256, 2048]
        for ch in range(8):
            out[256 * ch + 32 * c:256 * ch + 32 * c + 32, :] = \
                o_c[32 * ch:32 * ch + 32, :]
    return out.reshape(1, S, HID)


# revision 11
# speedup vs baseline: 1.1013x; 1.0222x over previous
"""GQA attention (B=1, S=2048, H=2048, 32 q-heads / 8 kv-heads, hd=64)
on 8 Trainium2 NeuronCores.

Sharding: tensor-parallel over heads. Core c owns q-heads 4c..4c+3 and
kv-head c: wq/wk/wv column shards, wo row shard; each core computes a
full [S, H] partial of the output projection; chunked bf16
ReduceScatters (256 output rows each) sum the partials; the host
scatters the per-core slices back together.

v3: all matmul operands bf16 (psum f32), software-pipelined per
512-column chunk c:
  A_c  projections (wq0/wq1/wkv stationary, persistent xT tiles moving)
  B_c  per-head RMSNorm: the 3 stats matmuls write one [66,512] psum
       tile at partition offsets 0/32/64, so a single Ln and a single
       Exp cover all of q0/q1/kv (2 act-table loads per chunk) + RoPE
       (partition-shift sbuf DMA issued from GpSimd) + V transpose
  C_c  causal attention for q-chunk c (scores^T -> exp -> P^T V_aug,
       split per kv-head for 1-bank psum tiles), 1/l via
       reciprocal_approx_fast, o_proj, ReduceScatter every 256 rows.
A_{c+1}'s matmuls are emitted as fillers interleaved into B_c/C_c so
TensorE never idles (keeps the HAM clock-gate at 2.4 GHz).
"""
import os
import sys

sys.path.insert(0, "/opt/trn_rl_repo")

import numpy as np  # noqa: E402
import ml_dtypes  # noqa: E402
import concourse.bacc as bacc  # noqa: E402
import concourse.mybir as mybir  # noqa: E402
import concourse.tile as tile  # noqa: E402
from concourse import bass_utils  # noqa: E402

f32 = mybir.dt.float32
bf16 = mybir.dt.bfloat16
AF = mybir.ActivationFunctionType
BF = ml_dtypes.bfloat16

N_CORES = 8
S = 2048
HID = 2048
HD = 64
ROPE_THETA = 10000.0
RMS_EPS = 1e-6
SCALING = HD ** -0.5              # 0.125
NK = HID // 128                   # 16 contraction tiles
NCH = 4                           # 512-col chunks

_NC_CACHE = None
LAST_RESULTS = None


def _build():
    nc = bacc.Bacc("TRN2", target_bir_lowering=False, debug=False,
                   num_devices=N_CORES)

    def din(name, shape, dt):
        return nc.dram_tensor(name, shape, dt, kind="ExternalInput").ap()

    xT = din("xT", [HID, S], bf16)
    # host-pretiled: row p, col block t = original rows 128t+p
    wq0 = din("wq0", [128, HID], bf16)
    wq1 = din("wq1", [128, HID], bf16)
    wkv = din("wkv", [128, HID], bf16)     # [wv | wk] columns pretiled
    wo0 = din("wo0", [128, S], bf16)
    wo1 = din("wo1", [128, S], bf16)
    cos2 = din("cos2", [128, S], bf16)
    ss2 = din("ss2", [128, S], bf16)
    ew_all = din("ew_all", [66, 128], bf16)
    e2 = din("e2", [2, 128], bf16)
    e2t = din("e2t", [128, 66], bf16)
    mask = din("mask", [128, 128], bf16)
    ident = din("ident", [64, 64], f32)

    out_rs = nc.dram_tensor("out_rs", [S // N_CORES, S], bf16,
                            kind="ExternalOutput").ap()

    with tile.TileContext(nc) as tc:
        with tc.tile_pool(name="consts", bufs=1) as cp, \
             tc.tile_pool(name="dram", bufs=1, space="DRAM") as dp:
            c_wq0 = cp.tile([128, HID], bf16, tag="w")
            c_wq1 = cp.tile([128, HID], bf16, tag="w2")
            c_wkv = cp.tile([128, HID], bf16, tag="w3")
            c_wo0 = cp.tile([128, S], bf16, tag="w4")
            c_wo1 = cp.tile([128, S], bf16, tag="w5")
            c_cos = cp.tile([128, S], bf16, tag="c1")
            c_ss = cp.tile([128, S], bf16, tag="c2")
            c_ew = cp.tile([66, 128], bf16, tag="c3")
            c_e2 = cp.tile([2, 128], bf16, tag="c5")
            c_e2t = cp.tile([128, 66], bf16, tag="c5t")
            c_mask = cp.tile([128, 128], bf16, tag="c6")
            c_id = cp.tile([64, 64], f32, tag="c7")
            c_eps = cp.tile([66, 1], f32, tag="c8")
            xts = [cp.tile([128, S], bf16, tag=f"x{t}", name=f"xts{t}")
                   for t in range(NK)]

            # weights column-block first: matmul ktile t only needs
            # col block t//4, so the first matmul unblocks after ~3
            # small DMAs instead of the whole 1.5 MB
            for cb in range(4):
                cs = slice(512 * cb, 512 * cb + 512)
                nc.sync.dma_start(c_wq0[:, cs], wq0[:, cs])
                nc.sync.dma_start(c_wq1[:, cs], wq1[:, cs])
                nc.sync.dma_start(c_wkv[:, cs], wkv[:, cs])
            nc.vector.memset(c_eps[:], RMS_EPS)
            nc.sync.dma_start(c_e2t[:], e2t)
            nc.sync.dma_start(c_ew[:], ew_all)
            nc.sync.dma_start(c_id[:], ident)
            nc.sync.dma_start(c_e2[:], e2)
            nc.sync.dma_start(c_mask[:], mask)
            # x row-blocks, first halves first (chunks 0/1 only touch
            # cols 0:1024); contiguous 2KB partition lines
            for t in range(NK):
                nc.sync.dma_start(xts[t][:, 0:1024],
                                  xT[128 * t:128 * t + 128, 0:1024])
                if t == 3:
                    nc.sync.dma_start(c_cos[:], cos2)
                    nc.sync.dma_start(c_ss[:], ss2)
            for h in range(2):
                hr = slice(64 * h, 64 * h + 64)
                nc.sync.dma_start(c_wo0[hr, :], wo0[hr, :])
                nc.sync.dma_start(c_wo1[hr, :], wo1[hr, :])
            for t in range(NK):
                nc.sync.dma_start(xts[t][:, 1024:2048],
                                  xT[128 * t:128 * t + 128, 1024:2048])

            qkv = {
                "q0": cp.tile([128, S], f32, tag="q0", name="q0"),
                "q1": cp.tile([128, S], f32, tag="q1", name="q1"),
                "kv": cp.tile([128, S], f32, tag="kv", name="kv"),
            }
            qr0 = cp.tile([128, S], bf16, tag="qr0")
            qr1 = cp.tile([128, S], bf16, tag="qr1")
            krd = cp.tile([128, S], bf16, tag="krd")
            v_aug = cp.tile([128, NK * (HD + 1)], bf16, tag="vaug")

            partial = dp.tile([S, S], bf16)
            rs_out = dp.tile([S // N_CORES, S], bf16)

            with tc.tile_pool(name="sbB", bufs=2) as sbB, \
                 tc.tile_pool(name="sbC", bufs=2) as sbC, \
                 tc.tile_pool(name="psA", bufs=3, space="PSUM") as psA, \
                 tc.tile_pool(name="psS", bufs=2, space="PSUM") as psS, \
                 tc.tile_pool(name="psPV", bufs=2, space="PSUM") as psPV, \
                 tc.tile_pool(name="psB", bufs=1, space="PSUM") as psB:

                # si 0 = kv (write rows 64:128 -> partition base 0),
                # si 1 = q0, si 2 = q1
                specs = [
                    ("kv", krd, True),
                    ("q0", qr0, False),
                    ("q1", qr1, False),
                ]

                def gen_A(c):
                    """Projection chunk c as a resumable generator:
                    one yield per ktile + two for the psum drains."""
                    cs = slice(512 * c, 512 * c + 512)
                    pa = [psA.tile([128, 512], f32, tag="pa",
                                   name=f"pa{c}_{j}") for j in range(3)]
                    for t in range(NK):
                        tc_ = slice(128 * t, 128 * (t + 1))
                        st = (t == 0)
                        sp = (t == NK - 1)
                        nc.tensor.matmul(pa[0][:], c_wq0[:, tc_],
                                         xts[t][:, cs], start=st, stop=sp)
                        nc.tensor.matmul(pa[1][:], c_wq1[:, tc_],
                                         xts[t][:, cs], start=st, stop=sp)
                        nc.tensor.matmul(pa[2][:], c_wkv[:, tc_],
                                         xts[t][:, cs], start=st, stop=sp)
                        yield
                    nc.vector.tensor_copy(qkv["q0"][:, cs], pa[0][:])
                    yield
                    nc.scalar.copy(qkv["q1"][:, cs], pa[1][:])
                    nc.vector.tensor_copy(qkv["kv"][:, cs], pa[2][:])
                    yield

                filler = [None]

                def pop(n=1):
                    g = filler[0]
                    if g is None:
                        return
                    for _ in range(n):
                        try:
                            next(g)
                        except StopIteration:
                            filler[0] = None
                            return

                def drain():
                    pop(NK + 3)

                def emit_B(c):
                    cs = slice(512 * c, 512 * c + 512)
                    if c == 0:
                        nc.gpsimd.memset(v_aug[:], 1.0)
                    # fused stats: one [66,512] psum tile, partition
                    # base 32*si per spec -> single Ln + single Exp
                    pss = psB.tile([66, 512], f32, tag="m",
                                   name=f"ss{c}")
                    for si, (key, dst, is_kv) in enumerate(specs):
                        sq = sbB.tile([128, 512], bf16, tag="sq", bufs=2,
                                      name=f"sq{c}_{si}")
                        nc.vector.tensor_mul(sq[:], qkv[key][:, cs],
                                             qkv[key][:, cs])
                        nc.tensor.matmul(pss[32 * si:32 * si + 2, :],
                                         c_e2t[:, 32 * si:32 * si + 2],
                                         sq[:], start=True, stop=True)
                        pop()
                    lnv = sbB.tile([66, 512], f32, tag="lnv", bufs=2,
                                   name=f"lnv{c}")
                    nc.scalar.activation(lnv[:], pss[:], AF.Ln,
                                         scale=1.0 / HD, bias=c_eps[:])
                    rstd = sbB.tile([66, 512], bf16, tag="rstd", bufs=2,
                                    name=f"rr{c}")
                    nc.scalar.activation(rstd[:], lnv[:],
                                         AF.Exp, scale=-0.5)
                    pop()
                    for si, (key, dst, is_kv) in enumerate(specs):
                        rows = slice(64, 128) if is_kv else slice(0, 128)
                        ps_ = slice(32 * si, 32 * si + 2)
                        pb = psB.tile([128, 512], f32, tag="m",
                                      name=f"pb{c}_{si}")
                        nc.tensor.matmul(pb[:], c_ew[ps_, :],
                                         rstd[ps_, :],
                                         start=True, stop=True)
                        nrm = sbB.tile([128, 512], bf16, tag="nrm",
                                       bufs=2, name=f"nrm{c}_{si}")
                        nc.vector.tensor_mul(nrm[rows, :],
                                             qkv[key][rows, cs],
                                             pb[rows, :])
                        # rope: rotate-half via partition-shift DMA
                        sh = sbB.tile([128, 512], bf16, tag="sh",
                                      bufs=2, name=f"sh{c}_{si}")
                        if is_kv:
                            nc.sync.dma_start(sh[64:96, :], nrm[96:128, :])
                            nc.sync.dma_start(sh[96:128, :], nrm[64:96, :])
                        else:
                            nc.sync.dma_start(sh[0:32, :], nrm[32:64, :])
                            nc.sync.dma_start(sh[32:64, :], nrm[0:32, :])
                            nc.sync.dma_start(sh[64:96, :], nrm[96:128, :])
                            nc.sync.dma_start(sh[96:128, :], nrm[64:96, :])
                        t2 = sbB.tile([128, 512], bf16, tag="t2",
                                      bufs=2, name=f"t2{c}_{si}")
                        nc.vector.tensor_mul(t2[rows, :], sh[rows, :],
                                             c_ss[rows, cs])
                        t1 = sbB.tile([128, 512], bf16, tag="t1",
                                      bufs=2, name=f"t1{c}_{si}")
                        nc.vector.tensor_mul(t1[rows, :], nrm[rows, :],
                                             c_cos[rows, cs])
                        nc.vector.tensor_add(dst[rows, cs], t1[rows, :],
                                             t2[rows, :])
                        pop()
                        if is_kv:
                            nc.sync.dma_start(dst[0:64, cs],
                                              dst[64:128, cs])
                            for tt in range(4 * c, 4 * c + 4):
                                ptr = psB.tile([128, 64], f32, tag="m",
                                               name=f"pt{c}_{tt}")
                                nc.tensor.transpose(
                                    ptr[:],
                                    qkv["kv"][0:64,
                                              128 * tt:128 * (tt + 1)],
                                    c_id[:])
                                nc.vector.tensor_copy(
                                    v_aug[:, (HD + 1) * tt:
                                          (HD + 1) * tt + HD],
                                    ptr[:])
                                pop()

                def emit_C(c):
                    attnf = []
                    lsb = []
                    ntile = 4 * c + 4
                    for hp, qr in ((0, qr0), (1, qr1)):
                        ppv_a = psPV.tile([65, 512], f32, tag="pv",
                                          name=f"pva{c}_{hp}")
                        ppv_b = psPV.tile([65, 512], f32, tag="pv",
                                          name=f"pvb{c}_{hp}")
                        for t in range(ntile):
                            r = t - 4 * c
                            off = max(0, r) * 128
                            qlo = 512 * c + off
                            qlen = 512 * (c + 1) - qlo
                            kc = slice(128 * t, 128 * (t + 1))
                            vs = slice((HD + 1) * t,
                                       (HD + 1) * t + HD + 1)
                            st = (t == 0)
                            sp = (t == ntile - 1)
                            pop()
                            ps_a = psS.tile([128, 512], f32, tag="s",
                                            name=f"sa{c}{hp}{t}")
                            nc.tensor.matmul(
                                ps_a[:, 0:qlen], krd[0:64, kc],
                                qr[0:64, qlo:qlo + qlen],
                                start=True, stop=True)
                            ps_b = psS.tile([128, 512], f32, tag="s",
                                            name=f"sb{c}{hp}{t}")
                            nc.tensor.matmul(
                                ps_b[:, 0:qlen], krd[64:128, kc],
                                qr[64:128, qlo:qlo + qlen],
                                start=True, stop=True)
                            pt_a = sbC.tile([128, 512], bf16, tag="pt",
                                            bufs=4, name=f"pa{c}{hp}{t}")
                            nc.scalar.activation(pt_a[:, 0:qlen],
                                                 ps_a[:, 0:qlen],
                                                 AF.Exp, scale=SCALING)
                            if r >= 0:
                                nc.vector.tensor_mul(
                                    pt_a[:, 0:128], pt_a[:, 0:128],
                                    c_mask[:])
                            nc.tensor.matmul(
                                ppv_a[:, off:512], v_aug[:, vs],
                                pt_a[:, 0:qlen], start=st, stop=sp)
                            pt_b = sbC.tile([128, 512], bf16, tag="pt",
                                            bufs=4, name=f"pb{c}{hp}{t}")
                            nc.scalar.activation(pt_b[:, 0:qlen],
                                                 ps_b[:, 0:qlen],
                                                 AF.Exp, scale=SCALING)
                            if r >= 0:
                                nc.vector.tensor_mul(
                                    pt_b[:, 0:128], pt_b[:, 0:128],
                                    c_mask[:])
                            nc.tensor.matmul(
                                ppv_b[:, off:512], v_aug[:, vs],
                                pt_b[:, 0:qlen], start=st, stop=sp)
                        # drain ppv -> attn rows + l row
                        af = sbC.tile([128, 512], f32, tag="attnf",
                                      bufs=2, name=f"af{c}_{hp}")
                        ls = sbC.tile([2, 512], f32, tag="l", bufs=2,
                                      name=f"ls{c}_{hp}")
                        for half, ppv in ((0, ppv_a), (1, ppv_b)):
                            stg = sbC.tile([65, 512], f32, tag="stg",
                                           bufs=2, name=f"st{c}{hp}{half}")
                            if half == 0:
                                nc.vector.tensor_copy(stg[:], ppv[:])
                            else:
                                nc.scalar.copy(stg[:], ppv[:])
                            nc.sync.dma_start(
                                af[64 * half:64 * half + 64, :],
                                stg[0:64, :])
                            nc.sync.dma_start(
                                ls[half:half + 1, :], stg[64:65, :])
                            pop()
                        attnf.append(af)
                        lsb.append(ls)
                    # normalize by 1/l, cast to bf16 for o_proj
                    attnb = []
                    for i in range(2):
                        rl = sbC.tile([2, 512], f32, tag="rl", bufs=2,
                                      name=f"rl{c}{i}")
                        nc.vector.reciprocal_approx_fast(rl[:], lsb[i][:])
                        rlb = sbC.tile([2, 512], bf16, tag="rlb", bufs=2,
                                       name=f"rlb{c}{i}")
                        nc.vector.tensor_copy(rlb[:], rl[:])
                        pb = psPV.tile([128, 512], f32, tag="pv",
                                       name=f"plb{c}{i}")
                        nc.tensor.matmul(pb[:], c_e2[:], rlb[:],
                                         start=True, stop=True)
                        ab = sbC.tile([128, 512], bf16, tag="attnb",
                                      bufs=2, name=f"ab{c}_{i}")
                        nc.vector.tensor_mul(ab[:], attnf[i][:], pb[:])
                        attnb.append(ab)
                        pop()
                    # o_proj rows + 256-row chunked reduce-scatter
                    for ml in range(4):
                        m = 4 * c + ml
                        mls = slice(128 * ml, 128 * ml + 128)
                        ost = sbC.tile([128, S], bf16, tag="ost", bufs=2,
                                       name=f"ost{c}_{ml}")
                        for n in range(4):
                            ns = slice(512 * n, 512 * n + 512)
                            po = psPV.tile([128, 512], f32, tag="pv",
                                           name=f"po{c}{ml}{n}")
                            nc.tensor.matmul(
                                po[:], attnb[0][:, mls], c_wo0[:, ns],
                                start=True, stop=False)
                            nc.tensor.matmul(
                                po[:], attnb[1][:, mls], c_wo1[:, ns],
                                start=False, stop=True)
                            if n % 2 == 0:
                                nc.vector.tensor_copy(ost[:, ns], po[:])
                            else:
                                nc.scalar.copy(ost[:, ns], po[:])
                            pop()
                        nc.sync.dma_start(
                            partial[128 * m:128 * (m + 1), :], ost[:])
                        # 128-row reduce-scatter: each core receives 16
                        # output rows per chunk; flows during compute,
                        # minimal tail after the last o_proj tile
                        nc.gpsimd.collective_compute(
                            "ReduceScatter",
                            mybir.AluOpType.add,
                            replica_groups=[list(range(N_CORES))],
                            ins=[partial[128 * m:128 * (m + 1),
                                         :].opt()],
                            outs=[rs_out[16 * m:16 * m + 16, :].opt()],
                        )
                        nc.sync.dma_start(
                            out_rs[16 * m:16 * m + 16, :],
                            rs_out[16 * m:16 * m + 16, :])

                # A_0 runs undeferred; each later A chunk drips into
                # the previous B/C as fillers
                for _ in gen_A(0):
                    pass
                for c in range(NCH):
                    if c + 1 < NCH:
                        filler[0] = gen_A(c + 1)
                    emit_B(c)
                    emit_C(c)
                    drain()

    nc.compile()
    return nc


def _host_prep(hidden_states, position_ids, wq, wk, wv, wo, q_ln_w, k_ln_w):
    x = np.asarray(hidden_states, dtype=np.float32)[0]        # [S, HID]
    xT = np.ascontiguousarray(x.T).astype(BF)                 # [HID, S]
    pos = np.asarray(position_ids)[0].astype(np.float32)      # [S]
    inv = 1.0 / (ROPE_THETA ** (np.arange(0, HD, 2, dtype=np.float32) / HD))
    ang = pos[:, None] * inv[None, :]                         # [S, 32]
    emb = np.concatenate([ang, ang], axis=1)                  # [S, 64]
    cosT = np.cos(emb).T.astype(np.float32)                   # [64, S]
    sinT = np.sin(emb).T.astype(np.float32)
    ss = sinT.copy()
    ss[0:32] = -sinT[0:32]
    cos2 = np.tile(cosT, (2, 1)).astype(BF)
    ss2 = np.tile(ss, (2, 1)).astype(BF)

    e2 = np.zeros((2, 128), dtype=np.float32)
    e2[0, 0:64] = 1.0
    e2[1, 64:128] = 1.0
    # fused stats layout: si 0 (kv) at partitions 0:2, si 1 (q0) at
    # 32:34, si 2 (q1) at 64:66; row pairs select [rows 0:64, 64:128]
    ew_all = np.zeros((66, 128), dtype=np.float32)
    ew_all[1, 64:128] = k_ln_w          # kv: k rows live at 64:128
    ew_all[32, 0:64] = q_ln_w
    ew_all[33, 64:128] = q_ln_w
    ew_all[64, 0:64] = q_ln_w
    ew_all[65, 64:128] = q_ln_w
    e2t = np.zeros((128, 66), dtype=np.float32)
    e2t[0:64, 0] = 1.0
    e2t[64:128, 1] = 1.0
    e2t[0:64, 32] = 1.0
    e2t[64:128, 33] = 1.0
    e2t[0:64, 64] = 1.0
    e2t[64:128, 65] = 1.0
    msk = (np.arange(128)[:, None] <= np.arange(128)[None, :]) \
        .astype(BF)
    ident = np.eye(64, dtype=np.float32)

    wq_ = np.asarray(wq, dtype=np.float32)
    wk_ = np.asarray(wk, dtype=np.float32)
    wv_ = np.asarray(wv, dtype=np.float32)
    wo_ = np.asarray(wo, dtype=np.float32)

    def pretile(w):  # [HID, 128] -> [128, HID] ktile-blocked
        return np.ascontiguousarray(
            w.reshape(NK, 128, 128).transpose(1, 0, 2).reshape(128, HID)
        ).astype(BF)

    in_maps = []
    for c in range(N_CORES):
        qcols = slice(256 * c, 256 * (c + 1))
        kvcols = slice(64 * c, 64 * (c + 1))
        wq_c = np.ascontiguousarray(wq_[:, qcols])
        wkv_c = np.concatenate([wv_[:, kvcols], wk_[:, kvcols]], axis=1)
        wo_c = np.ascontiguousarray(wo_[256 * c:256 * (c + 1), :])
        in_maps.append({
            "xT": xT,
            "wq0": pretile(wq_c[:, 0:128]),
            "wq1": pretile(wq_c[:, 128:256]),
            "wkv": pretile(wkv_c),
            "wo0": np.ascontiguousarray(wo_c[0:128, :]).astype(BF),
            "wo1": np.ascontiguousarray(wo_c[128:256, :]).astype(BF),
            "cos2": cos2,
            "ss2": ss2,
            "ew_all": ew_all.astype(BF),
            "e2": e2.astype(BF),
            "e2t": np.ascontiguousarray(e2t).astype(BF),
            "mask": msk,
            "ident": ident,
        })
    return in_maps


def kernel(hidden_states, position_ids, wq, wk, wv, wo, q_ln_w, k_ln_w):
    global _NC_CACHE, LAST_RESULTS
    if _NC_CACHE is None:
        _NC_CACHE = _build()
    nc = _NC_CACHE
    in_maps = _host_prep(hidden_states, position_ids, wq, wk, wv, wo,
                         q_ln_w, k_ln_w)
    res = bass_utils.run_bass_kernel_spmd(
        nc, in_maps, core_ids=list(range(N_CORES)))
    LAST_RESULTS = res
    out = np.empty((S, HID), dtype=np.float32)
    for c in range(N_CORES):
        # [256, 2048] bf16; RS chunk j covered partial rows
        # [128j, 128j+128) and left core c rows [128j+16c, 128j+16c+16)
        o_c = np.asarray(res.results[c]["out_rs"]).astype(np.float32)
        for j in range(16):
            out[128 * j + 16 * c:128 * j + 16 * c + 16, :] = \
                o_c[16 * j:16 * j + 16, :]
    return out.reshape(1, S, HID)


# revision 14
# speedup vs baseline: 1.2149x; 1.1031x over previous
"""GQA attention (B=1, S=2048, H=2048, 32 q-heads / 8 kv-heads, hd=64)
on 8 Trainium2 NeuronCores.

Sharding: tensor-parallel over heads. Core c owns q-heads 4c..4c+3 and
kv-head c: wq/wk/wv column shards, wo row shard; each core computes a
full [S, H] partial of the output projection; chunked bf16
ReduceScatters (256 output rows each) sum the partials; the host
scatters the per-core slices back together.

v3: all matmul operands bf16 (psum f32), software-pipelined per
512-column chunk c:
  A_c  projections (wq0/wq1/wkv stationary, persistent xT tiles moving)
  B_c  per-head RMSNorm: the 3 stats matmuls write one [66,512] psum
       tile at partition offsets 0/32/64, so a single Ln and a single
       Exp cover all of q0/q1/kv (2 act-table loads per chunk) + RoPE
       (partition-shift sbuf DMA issued from GpSimd) + V transpose
  C_c  causal attention for q-chunk c (scores^T -> exp -> P^T V_aug,
       split per kv-head for 1-bank psum tiles), 1/l via
       reciprocal_approx_fast, o_proj, ReduceScatter every 256 rows.
A_{c+1}'s matmuls are emitted as fillers interleaved into B_c/C_c so
TensorE never idles (keeps the HAM clock-gate at 2.4 GHz).
"""
import os
import sys

sys.path.insert(0, "/opt/trn_rl_repo")

import numpy as np  # noqa: E402
import ml_dtypes  # noqa: E402
import concourse.bacc as bacc  # noqa: E402
import concourse.mybir as mybir  # noqa: E402
import concourse.tile as tile  # noqa: E402
from concourse import bass_utils  # noqa: E402

f32 = mybir.dt.float32
bf16 = mybir.dt.bfloat16
AF = mybir.ActivationFunctionType
BF = ml_dtypes.bfloat16

N_CORES = 8
S = 2048
HID = 2048
HD = 64
ROPE_THETA = 10000.0
RMS_EPS = 1e-6
SCALING = HD ** -0.5              # 0.125
NK = HID // 128                   # 16 contraction tiles
NCH = 4                           # 512-col chunks

_NC_CACHE = None
LAST_RESULTS = None


def _build():
    nc = bacc.Bacc("TRN2", target_bir_lowering=False, debug=False,
                   num_devices=N_CORES)

    def din(name, shape, dt):
        return nc.dram_tensor(name, shape, dt, kind="ExternalInput").ap()

    xT = din("xT", [HID, S], bf16)
    # host-pretiled: row p, col block t = original rows 128t+p
    wq0 = din("wq0", [128, HID], bf16)
    wq1 = din("wq1", [128, HID], bf16)
    wkv = din("wkv", [128, HID], bf16)     # [wv | wk] columns pretiled
    wo0 = din("wo0", [128, S], bf16)
    wo1 = din("wo1", [128, S], bf16)
    cos2 = din("cos2", [128, S], bf16)
    ss2 = din("ss2", [128, S], bf16)
    ew_all = din("ew_all", [66, 128], bf16)
    e2 = din("e2", [2, 128], bf16)
    e2t = din("e2t", [128, 66], bf16)
    mask = din("mask", [128, 128], bf16)
    ident = din("ident", [64, 64], f32)

    out_rs = nc.dram_tensor("out_rs", [S // N_CORES, S], bf16,
                            kind="ExternalOutput").ap()

    with tile.TileContext(nc) as tc:
        with tc.tile_pool(name="consts", bufs=1) as cp, \
             tc.tile_pool(name="dram", bufs=1, space="DRAM") as dp:
            c_wq0 = cp.tile([128, HID], bf16, tag="w")
            c_wq1 = cp.tile([128, HID], bf16, tag="w2")
            c_wkv = cp.tile([128, HID], bf16, tag="w3")
            c_wo0 = cp.tile([128, S], bf16, tag="w4")
            c_wo1 = cp.tile([128, S], bf16, tag="w5")
            c_cos = cp.tile([128, S], bf16, tag="c1")
            c_ss = cp.tile([128, S], bf16, tag="c2")
            c_ew = cp.tile([66, 128], bf16, tag="c3")
            c_e2 = cp.tile([2, 128], bf16, tag="c5")
            c_e2t = cp.tile([128, 66], bf16, tag="c5t")
            c_mask = cp.tile([128, 128], bf16, tag="c6")
            c_id = cp.tile([64, 64], f32, tag="c7")
            c_eps = cp.tile([66, 1], f32, tag="c8")
            xts = [cp.tile([128, S], bf16, tag=f"x{t}", name=f"xts{t}")
                   for t in range(NK)]

            # weights column-block first: matmul ktile t only needs
            # col block t//4, so the first matmul unblocks after ~3
            # small DMAs instead of the whole 1.5 MB
            for cb in range(4):
                cs = slice(512 * cb, 512 * cb + 512)
                nc.sync.dma_start(c_wq0[:, cs], wq0[:, cs])
                nc.sync.dma_start(c_wq1[:, cs], wq1[:, cs])
                nc.sync.dma_start(c_wkv[:, cs], wkv[:, cs])
            nc.vector.memset(c_eps[:], RMS_EPS)
            nc.sync.dma_start(c_e2t[:], e2t)
            nc.sync.dma_start(c_ew[:], ew_all)
            nc.sync.dma_start(c_id[:], ident)
            nc.sync.dma_start(c_e2[:], e2)
            nc.sync.dma_start(c_mask[:], mask)
            # x row-blocks, chunk-0's quarter first so A_0 starts on
            # minimal critical bytes, then the rest
            for t in range(NK):
                nc.sync.dma_start(xts[t][:, 0:512],
                                  xT[128 * t:128 * t + 128, 0:512])
            nc.sync.dma_start(c_cos[:], cos2)
            nc.sync.dma_start(c_ss[:], ss2)
            for t in range(NK):
                nc.sync.dma_start(xts[t][:, 512:1024],
                                  xT[128 * t:128 * t + 128, 512:1024])
            for h in range(2):
                hr = slice(64 * h, 64 * h + 64)
                nc.sync.dma_start(c_wo0[hr, :], wo0[hr, :])
                nc.sync.dma_start(c_wo1[hr, :], wo1[hr, :])
            for t in range(NK):
                nc.sync.dma_start(xts[t][:, 1024:2048],
                                  xT[128 * t:128 * t + 128, 1024:2048])

            qkv = {
                "q0": cp.tile([128, S], f32, tag="q0", name="q0"),
                "q1": cp.tile([128, S], f32, tag="q1", name="q1"),
                "kv": cp.tile([128, S], f32, tag="kv", name="kv"),
            }
            qr0 = cp.tile([128, S], bf16, tag="qr0")
            qr1 = cp.tile([128, S], bf16, tag="qr1")
            krd = cp.tile([128, S], bf16, tag="krd")
            v_aug = cp.tile([128, NK * (HD + 1)], bf16, tag="vaug")

            partial = dp.tile([S, S], bf16)
            rs_out = dp.tile([S // N_CORES, S], bf16)

            with tc.tile_pool(name="sbB", bufs=2) as sbB, \
                 tc.tile_pool(name="sbC", bufs=2) as sbC, \
                 tc.tile_pool(name="psA", bufs=3, space="PSUM") as psA, \
                 tc.tile_pool(name="psS", bufs=2, space="PSUM") as psS, \
                 tc.tile_pool(name="psPV", bufs=2, space="PSUM") as psPV, \
                 tc.tile_pool(name="psB", bufs=1, space="PSUM") as psB:

                # si 0 = kv (write rows 64:128 -> partition base 0),
                # si 1 = q0, si 2 = q1
                specs = [
                    ("kv", krd, True),
                    ("q0", qr0, False),
                    ("q1", qr1, False),
                ]

                def gen_A(c):
                    """Projection chunk c as a resumable generator:
                    one yield per ktile + two for the psum drains."""
                    cs = slice(512 * c, 512 * c + 512)
                    pa = [psA.tile([128, 512], f32, tag="pa",
                                   name=f"pa{c}_{j}") for j in range(3)]
                    for t in range(NK):
                        tc_ = slice(128 * t, 128 * (t + 1))
                        st = (t == 0)
                        sp = (t == NK - 1)
                        nc.tensor.matmul(pa[0][:], c_wq0[:, tc_],
                                         xts[t][:, cs], start=st, stop=sp)
                        nc.tensor.matmul(pa[1][:], c_wq1[:, tc_],
                                         xts[t][:, cs], start=st, stop=sp)
                        nc.tensor.matmul(pa[2][:], c_wkv[:, tc_],
                                         xts[t][:, cs], start=st, stop=sp)
                        yield
                    nc.vector.tensor_copy(qkv["q0"][:, cs], pa[0][:])
                    yield
                    nc.scalar.copy(qkv["q1"][:, cs], pa[1][:])
                    nc.vector.tensor_copy(qkv["kv"][:, cs], pa[2][:])
                    yield

                filler = [None]

                def pop(n=1):
                    g = filler[0]
                    if g is None:
                        return
                    for _ in range(n):
                        try:
                            next(g)
                        except StopIteration:
                            filler[0] = None
                            return

                def drain():
                    pop(NK + 3)

                def emit_B(c):
                    cs = slice(512 * c, 512 * c + 512)
                    if c == 0:
                        nc.gpsimd.memset(v_aug[:], 1.0)
                    # fused stats: one [66,512] psum tile, partition
                    # base 32*si per spec -> single Ln + single Exp
                    pss = psB.tile([66, 512], f32, tag="m",
                                   name=f"ss{c}")
                    for si, (key, dst, is_kv) in enumerate(specs):
                        sq = sbB.tile([128, 512], bf16, tag="sq", bufs=2,
                                      name=f"sq{c}_{si}")
                        nc.vector.tensor_mul(sq[:], qkv[key][:, cs],
                                             qkv[key][:, cs])
                        nc.tensor.matmul(pss[32 * si:32 * si + 2, :],
                                         c_e2t[:, 32 * si:32 * si + 2],
                                         sq[:], start=True, stop=True)
                        pop()
                    lnv = sbB.tile([66, 512], f32, tag="lnv", bufs=2,
                                   name=f"lnv{c}")
                    nc.scalar.activation(lnv[:], pss[:], AF.Ln,
                                         scale=1.0 / HD, bias=c_eps[:])
                    rstd = sbB.tile([66, 512], bf16, tag="rstd", bufs=2,
                                    name=f"rr{c}")
                    nc.scalar.activation(rstd[:], lnv[:],
                                         AF.Exp, scale=-0.5)
                    pop()
                    for si, (key, dst, is_kv) in enumerate(specs):
                        rows = slice(64, 128) if is_kv else slice(0, 128)
                        ps_ = slice(32 * si, 32 * si + 2)
                        pb = psB.tile([128, 512], f32, tag="m",
                                      name=f"pb{c}_{si}")
                        nc.tensor.matmul(pb[:], c_ew[ps_, :],
                                         rstd[ps_, :],
                                         start=True, stop=True)
                        nrm = sbB.tile([128, 512], bf16, tag="nrm",
                                       bufs=2, name=f"nrm{c}_{si}")
                        nc.vector.tensor_mul(nrm[rows, :],
                                             qkv[key][rows, cs],
                                             pb[rows, :])
                        # rope: rotate-half via partition-shift DMA
                        sh = sbB.tile([128, 512], bf16, tag="sh",
                                      bufs=2, name=f"sh{c}_{si}")
                        if is_kv:
                            nc.sync.dma_start(sh[64:96, :], nrm[96:128, :])
                            nc.sync.dma_start(sh[96:128, :], nrm[64:96, :])
                        else:
                            nc.sync.dma_start(sh[0:32, :], nrm[32:64, :])
                            nc.sync.dma_start(sh[32:64, :], nrm[0:32, :])
                            nc.sync.dma_start(sh[64:96, :], nrm[96:128, :])
                            nc.sync.dma_start(sh[96:128, :], nrm[64:96, :])
                        t2 = sbB.tile([128, 512], bf16, tag="t2",
                                      bufs=2, name=f"t2{c}_{si}")
                        nc.vector.tensor_mul(t2[rows, :], sh[rows, :],
                                             c_ss[rows, cs])
                        t1 = sbB.tile([128, 512], bf16, tag="t1",
                                      bufs=2, name=f"t1{c}_{si}")
                        nc.vector.tensor_mul(t1[rows, :], nrm[rows, :],
                                             c_cos[rows, cs])
                        nc.vector.tensor_add(dst[rows, cs], t1[rows, :],
                                             t2[rows, :])
                        pop()
                        if is_kv:
                            nc.sync.dma_start(dst[0:64, cs],
                                              dst[64:128, cs])
                            for tt in range(4 * c, 4 * c + 4):
                                ptr = psB.tile([128, 64], f32, tag="m",
                                               name=f"pt{c}_{tt}")
                                nc.tensor.transpose(
                                    ptr[:],
                                    qkv["kv"][0:64,
                                              128 * tt:128 * (tt + 1)],
                                    c_id[:])
                                nc.vector.tensor_copy(
                                    v_aug[:, (HD + 1) * tt:
                                          (HD + 1) * tt + HD],
                                    ptr[:])
                                pop()

                def emit_C(c):
                    attnf = []
                    lsb = []
                    ntile = 4 * c + 4
                    for hp, qr in ((0, qr0), (1, qr1)):
                        ppv_a = psPV.tile([65, 512], f32, tag="pv",
                                          name=f"pva{c}_{hp}")
                        ppv_b = psPV.tile([65, 512], f32, tag="pv",
                                          name=f"pvb{c}_{hp}")
                        for t in range(ntile):
                            r = t - 4 * c
                            off = max(0, r) * 128
                            qlo = 512 * c + off
                            qlen = 512 * (c + 1) - qlo
                            kc = slice(128 * t, 128 * (t + 1))
                            vs = slice((HD + 1) * t,
                                       (HD + 1) * t + HD + 1)
                            st = (t == 0)
                            sp = (t == ntile - 1)
                            pop()
                            ps_a = psS.tile([128, 512], f32, tag="s",
                                            name=f"sa{c}{hp}{t}")
                            nc.tensor.matmul(
                                ps_a[:, 0:qlen], krd[0:64, kc],
                                qr[0:64, qlo:qlo + qlen],
                                start=True, stop=True)
                            ps_b = psS.tile([128, 512], f32, tag="s",
                                            name=f"sb{c}{hp}{t}")
                            nc.tensor.matmul(
                                ps_b[:, 0:qlen], krd[64:128, kc],
                                qr[64:128, qlo:qlo + qlen],
                                start=True, stop=True)
                            pt_a = sbC.tile([128, 512], bf16, tag="pt",
                                            bufs=4, name=f"pa{c}{hp}{t}")
                            nc.scalar.activation(pt_a[:, 0:qlen],
                                                 ps_a[:, 0:qlen],
                                                 AF.Exp, scale=SCALING)
                            if r >= 0:
                                nc.vector.tensor_mul(
                                    pt_a[:, 0:128], pt_a[:, 0:128],
                                    c_mask[:])
                            nc.tensor.matmul(
                                ppv_a[:, off:512], v_aug[:, vs],
                                pt_a[:, 0:qlen], start=st, stop=sp)
                            pt_b = sbC.tile([128, 512], bf16, tag="pt",
                                            bufs=4, name=f"pb{c}{hp}{t}")
                            nc.scalar.activation(pt_b[:, 0:qlen],
                                                 ps_b[:, 0:qlen],
                                                 AF.Exp, scale=SCALING)
                            if r >= 0:
                                nc.vector.tensor_mul(
                                    pt_b[:, 0:128], pt_b[:, 0:128],
                                    c_mask[:])
                            nc.tensor.matmul(
                                ppv_b[:, off:512], v_aug[:, vs],
                                pt_b[:, 0:qlen], start=st, stop=sp)
                        # drain ppv -> attn rows + l row
                        af = sbC.tile([128, 512], f32, tag="attnf",
                                      bufs=2, name=f"af{c}_{hp}")
                        ls = sbC.tile([2, 512], f32, tag="l", bufs=2,
                                      name=f"ls{c}_{hp}")
                        for half, ppv in ((0, ppv_a), (1, ppv_b)):
                            stg = sbC.tile([65, 512], f32, tag="stg",
                                           bufs=2, name=f"st{c}{hp}{half}")
                            if half == 0:
                                nc.vector.tensor_copy(stg[:], ppv[:])
                            else:
                                nc.scalar.copy(stg[:], ppv[:])
                            nc.sync.dma_start(
                                af[64 * half:64 * half + 64, :],
                                stg[0:64, :])
                            nc.sync.dma_start(
                                ls[half:half + 1, :], stg[64:65, :])
                            pop()
                        attnf.append(af)
                        lsb.append(ls)
                    # normalize by 1/l, cast to bf16 for o_proj
                    attnb = []
                    for i in range(2):
                        rl = sbC.tile([2, 512], f32, tag="rl", bufs=2,
                                      name=f"rl{c}{i}")
                        nc.vector.reciprocal_approx_fast(rl[:], lsb[i][:])
                        rlb = sbC.tile([2, 512], bf16, tag="rlb", bufs=2,
                                       name=f"rlb{c}{i}")
                        nc.vector.tensor_copy(rlb[:], rl[:])
                        pb = psPV.tile([128, 512], f32, tag="pv",
                                       name=f"plb{c}{i}")
                        nc.tensor.matmul(pb[:], c_e2[:], rlb[:],
                                         start=True, stop=True)
                        ab = sbC.tile([128, 512], bf16, tag="attnb",
                                      bufs=2, name=f"ab{c}_{i}")
                        nc.vector.tensor_mul(ab[:], attnf[i][:], pb[:])
                        attnb.append(ab)
                        pop()
                    # o_proj rows + 256-row chunked reduce-scatter
                    for ml in range(4):
                        m = 4 * c + ml
                        mls = slice(128 * ml, 128 * ml + 128)
                        ost = sbC.tile([128, S], bf16, tag="ost", bufs=2,
                                       name=f"ost{c}_{ml}")
                        for n in range(4):
                            ns = slice(512 * n, 512 * n + 512)
                            po = psPV.tile([128, 512], f32, tag="pv",
                                           name=f"po{c}{ml}{n}")
                            nc.tensor.matmul(
                                po[:], attnb[0][:, mls], c_wo0[:, ns],
                                start=True, stop=False)
                            nc.tensor.matmul(
                                po[:], attnb[1][:, mls], c_wo1[:, ns],
                                start=False, stop=True)
                            if n % 2 == 0:
                                nc.vector.tensor_copy(ost[:, ns], po[:])
                            else:
                                nc.scalar.copy(ost[:, ns], po[:])
                            pop()
                        nc.sync.dma_start(
                            partial[128 * m:128 * (m + 1), :], ost[:])
                        # reduce-scatter chunking: ~7.7us fixed cost
                        # per collective + ~9us/MB, so 384-row chunks
                        # through the body and 256-row chunks at the
                        # end (smaller tail after the last o_proj).
                        # chunk (m0, nm) covers partial rows
                        # [128*m0, 128*(m0+nm)); core c receives
                        # 16*nm rows at rs_out[16*m0 : 16*m0+16*nm]
                        if m in (2, 5, 8, 11, 13, 15):
                            nm = 2 if m >= 13 else 3
                            m0 = m - nm + 1
                            nc.gpsimd.collective_compute(
                                "ReduceScatter",
                                mybir.AluOpType.add,
                                replica_groups=[list(range(N_CORES))],
                                ins=[partial[128 * m0:128 * (m + 1),
                                             :].opt()],
                                outs=[rs_out[16 * m0:
                                             16 * m0 + 16 * nm,
                                             :].opt()],
                            )
                            nc.gpsimd.dma_start(
                                out_rs[16 * m0:16 * m0 + 16 * nm, :],
                                rs_out[16 * m0:16 * m0 + 16 * nm, :])

                # A_0 runs undeferred; each later A chunk drips into
                # the previous B/C as fillers
                for _ in gen_A(0):
                    pass
                for c in range(NCH):
                    if c + 1 < NCH:
                        filler[0] = gen_A(c + 1)
                    emit_B(c)
                    emit_C(c)
                    drain()

    nc.compile()
    return nc


def _host_prep(hidden_states, position_ids, wq, wk, wv, wo, q_ln_w, k_ln_w):
    x = np.asarray(hidden_states, dtype=np.float32)[0]        # [S, HID]
    xT = np.ascontiguousarray(x.T).astype(BF)                 # [HID, S]
    pos = np.asarray(position_ids)[0].astype(np.float32)      # [S]
    inv = 1.0 / (ROPE_THETA ** (np.arange(0, HD, 2, dtype=np.float32) / HD))
    ang = pos[:, None] * inv[None, :]                         # [S, 32]
    emb = np.concatenate([ang, ang], axis=1)                  # [S, 64]
    cosT = np.cos(emb).T.astype(np.float32)                   # [64, S]
    sinT = np.sin(emb).T.astype(np.float32)
    ss = sinT.copy()
    ss[0:32] = -sinT[0:32]
    cos2 = np.tile(cosT, (2, 1)).astype(BF)
    ss2 = np.tile(ss, (2, 1)).astype(BF)

    e2 = np.zeros((2, 128), dtype=np.float32)
    e2[0, 0:64] = 1.0
    e2[1, 64:128] = 1.0
    # fused stats layout: si 0 (kv) at partitions 0:2, si 1 (q0) at
    # 32:34, si 2 (q1) at 64:66; row pairs select [rows 0:64, 64:128]
    ew_all = np.zeros((66, 128), dtype=np.float32)
    ew_all[1, 64:128] = k_ln_w          # kv: k rows live at 64:128
    ew_all[32, 0:64] = q_ln_w
    ew_all[33, 64:128] = q_ln_w
    ew_all[64, 0:64] = q_ln_w
    ew_all[65, 64:128] = q_ln_w
    e2t = np.zeros((128, 66), dtype=np.float32)
    e2t[0:64, 0] = 1.0
    e2t[64:128, 1] = 1.0
    e2t[0:64, 32] = 1.0
    e2t[64:128, 33] = 1.0
    e2t[0:64, 64] = 1.0
    e2t[64:128, 65] = 1.0
    msk = (np.arange(128)[:, None] <= np.arange(128)[None, :]) \
        .astype(BF)
    ident = np.eye(64, dtype=np.float32)

    wq_ = np.asarray(wq, dtype=np.float32)
    wk_ = np.asarray(wk, dtype=np.float32)
    wv_ = np.asarray(wv, dtype=np.float32)
    wo_ = np.asarray(wo, dtype=np.float32)

    def pretile(w):  # [HID, 128] -> [128, HID] ktile-blocked
        return np.ascontiguousarray(
            w.reshape(NK, 128, 128).transpose(1, 0, 2).reshape(128, HID)
        ).astype(BF)

    in_maps = []
    for c in range(N_CORES):
        qcols = slice(256 * c, 256 * (c + 1))
        kvcols = slice(64 * c, 64 * (c + 1))
        wq_c = np.ascontiguousarray(wq_[:, qcols])
        wkv_c = np.concatenate([wv_[:, kvcols], wk_[:, kvcols]], axis=1)
        wo_c = np.ascontiguousarray(wo_[256 * c:256 * (c + 1), :])
        in_maps.append({
            "xT": xT,
            "wq0": pretile(wq_c[:, 0:128]),
            "wq1": pretile(wq_c[:, 128:256]),
            "wkv": pretile(wkv_c),
            "wo0": np.ascontiguousarray(wo_c[0:128, :]).astype(BF),
            "wo1": np.ascontiguousarray(wo_c[128:256, :]).astype(BF),
            "cos2": cos2,
            "ss2": ss2,
            "ew_all": ew_all.astype(BF),
            "e2": e2.astype(BF),
            "e2t": np.ascontiguousarray(e2t).astype(BF),
            "mask": msk,
            "ident": ident,
        })
    return in_maps


def kernel(hidden_states, position_ids, wq, wk, wv, wo, q_ln_w, k_ln_w):
    global _NC_CACHE, LAST_RESULTS
    if _NC_CACHE is None:
        _NC_CACHE = _build()
    nc = _NC_CACHE
    in_maps = _host_prep(hidden_states, position_ids, wq, wk, wv, wo,
                         q_ln_w, k_ln_w)
    res = bass_utils.run_bass_kernel_spmd(
        nc, in_maps, core_ids=list(range(N_CORES)))
    LAST_RESULTS = res
    out = np.empty((S, HID), dtype=np.float32)
    chunks = [(0, 3), (3, 3), (6, 3), (9, 3), (12, 2), (14, 2)]
    for c in range(N_CORES):
        # [256, 2048] bf16; RS chunk (m0, nm) covered partial rows
        # [128*m0, 128*(m0+nm)); core c got rows
        # [128*m0 + 16*nm*c, ... + 16*nm) at o_c[16*m0 : 16*m0+16*nm]
        o_c = np.asarray(res.results[c]["out_rs"]).astype(np.float32)
        for m0, nm in chunks:
            base = 128 * m0 + 16 * nm * c
            out[base:base + 16 * nm, :] = \
                o_c[16 * m0:16 * m0 + 16 * nm, :]
    return out.reshape(1, S, HID)
